# revision 28
# baseline (speedup 1.0000x reference)
"""EHM (SMPLX body + FLAME head + MANO hands) Bass kernel for 8 TRN2 NeuronCores.

Sharding: VERTEX sharding - each core owns 1/8 of the SMPLX vertices (plus the
FLAME/MANO vertices its SMPLX rows stitch in) and computes ALL B=128 batch
elements for its shard.

v3: identity-subtree folding + M-form FK.
  * SMPLX joints 22-54 have zero pose, so A_rel(j) == A_rel(posed ancestor).
    LBS weights fold on host: 55 joints -> 22.  FLAME neck/root and MANO root
    rows are the identity affine; jaw/eyes and hand-d1 joints need no chain
    composition at all.
  * A_rel(j) = A_rel(par) o M_j with M_j = [R_rel | (I-R_rel) J_j]: the rel-
    translation correction ("corr" pass) folds into the per-joint matrix.
  * One unified 64-row rhs [joint, (n4,m3,b)] feeds all skinning matmuls;
    scale/mirror/bias folds and the eyelid offsets live in virtual rows 55-63.
  * Arm chains split via pair-products (X = M_a o M_b) so Vector and GpSimd
    compose in parallel; hand levels are single strided runs.
  * Skinning tail: PSUM->fp16 copies split Scalar/GpSimd, applies split
    Vector/GpSimd.  Inputs stream per-chunk through separate DMA tiles.

Per-vertex data layout: [vertex(partition<=128), (c, b)] with c-major free dim
(col = c*128 + b).  Batch-staged data (poses, FK, A matrices): [b(part), free].
"""

import sys

sys.path.insert(0, "/opt/trn_rl_repo")

from contextlib import ExitStack

import numpy as np
import ml_dtypes

BF16NP = ml_dtypes.bfloat16
F16NP = np.float16

import concourse.bass as bass
import concourse.bacc as bacc
import concourse.tile as tile
import concourse.mybir as mybir
from concourse.bass_utils import run_bass_kernel_spmd

F32 = mybir.dt.float32
BF16 = mybir.dt.bfloat16
F16 = mybir.dt.float16
AF = mybir.ActivationFunctionType
ALU = mybir.AluOpType

# ---------------------------------------------------------------- constants
B = 128
VS, VF, VM = 10475, 5023, 778
NL = 350
NCORES = 8

SMPLX_PARENTS = np.array([-1,0,0,0,1,2,3,4,5,6,7,8,9,9,9,12,13,14,16,17,18,19,
                          15,15,15,20,25,26,20,28,29,20,31,32,20,34,35,20,37,38,
                          21,40,41,21,43,44,21,46,47,21,49,50,21,52,53])
MANO_PARENTS = np.array([-1,0,1,2,0,4,5,0,7,8,0,10,11,0,13,14])
HAND_LVL = [1,4,7,10,13, 2,5,8,11,14, 3,6,9,12,15]   # mano joints, level-major

N_PLAIN, N_HEAD, N_HL, N_HR = 768, 384, 128, 128
ROWS = N_PLAIN + N_HEAD + N_HL + N_HR        # 1408
NCH = ROWS // 128                            # 11
NCH_PLAIN = 6
CH_PLAIN = set(range(0, NCH_PLAIN))
CH_HEAD0 = 6                                 # chunks 6,7,8 head; 9 L; 10 R
CH_HL, CH_HR = 9, 10

PD_S_K = 189
PD_F_K = 27
PD_M_K = 135

# unified A/rhs row map (64 rows):
# 0:22 body | 22:25 flame jaw,eyeL,eyeR | 25:30 Ld1 | 30:35 Rd1 | 35:40 Ld2
# | 40:45 Rd2 | 45:50 Ld3 | 50:55 Rd3 | 55 flameId | 56 Lid | 57 Rid
# | 58:61 r_eyelid rows | 61:64 l_eyelid rows
NJ64 = 64

BF16_INPUTS = {"sd_s0", "sd_s1", "sd_s2", "pd_s_a", "pd_s_b", "sd_f", "pd_f",
               "sd_m", "pd_m_a", "pd_m_b", "cb_b"}
F16_INPUTS = {"w_s", "w_f", "w_m"}


# ================================================================ host prep

def _split_sizes(total, parts):
    q, r = divmod(total, parts)
    return [q + (1 if i < r else 0) for i in range(parts)]


def _pad_ids(ids, n):
    out = np.full(n, -1, np.int64)
    out[:len(ids)] = ids
    return out


def _host_prep(inp):
    f32 = np.float32
    s2f = np.asarray(inp["smplx2flame_ind"])
    head_ix = np.asarray(inp["head_index"])
    s2l = np.asarray(inp["smplx2mano_left"])
    s2r = np.asarray(inp["smplx2mano_right"])

    head_sv = s2f[head_ix]
    special = np.zeros(VS, bool)
    special[head_sv] = True
    special[s2l] = True
    special[s2r] = True
    plain_sv = np.nonzero(~special)[0]

    pl_sp = np.cumsum([0] + _split_sizes(len(plain_sv), NCORES))
    hd_sp = np.cumsum([0] + _split_sizes(len(head_ix), NCORES))
    hl_sp = np.cumsum([0] + _split_sizes(VM, NCORES))

    sd_s_np = np.asarray(inp["smplx_shapedirs"], f32)
    pd_s_np = np.asarray(inp["smplx_posedirs"], f32)
    jr_s_np = np.asarray(inp["smplx_J_regressor"], f32)
    w_s_np = np.asarray(inp["smplx_lbs_weights"], f32)
    tmpl_s = np.asarray(inp["smplx_v_template"], f32)
    sd_f_np = np.asarray(inp["flame_shapedirs"], f32)
    pd_f_np = np.asarray(inp["flame_posedirs"], f32)
    jr_f_np = np.asarray(inp["flame_J_regressor"], f32)
    w_f_np = np.asarray(inp["flame_lbs_weights"], f32)
    tmpl_f = np.asarray(inp["flame_v_template"], f32)
    re_np = np.asarray(inp["r_eyelid"], f32)
    le_np = np.asarray(inp["l_eyelid"], f32)
    sd_m_np = np.asarray(inp["mano_shapedirs"], f32)
    pd_m_np = np.asarray(inp["mano_posedirs"], f32)
    jr_m_np = np.asarray(inp["mano_J_regressor"], f32)
    w_m_np = np.asarray(inp["mano_lbs_weights"], f32)
    tmpl_m = np.asarray(inp["mano_v_template"], f32)

    # folded smplx weights: 55 -> 22
    w_fold = w_s_np[:, 0:22].copy()
    w_fold[:, 15] += w_s_np[:, 22:25].sum(1)
    w_fold[:, 20] += w_s_np[:, 25:40].sum(1)
    w_fold[:, 21] += w_s_np[:, 40:55].sum(1)

    # pose vector in v3 55-joint order
    lh = np.asarray(inp["left_hand_pose"], f32).reshape(B, 15, 3)
    rh = np.asarray(inp["right_hand_pose"], f32).reshape(B, 15, 3)
    lvl = [j - 1 for j in HAND_LVL]
    aa = np.concatenate([
        np.asarray(inp["global_pose"], f32).reshape(B, 3),
        np.asarray(inp["body_pose"], f32).reshape(B, 63),
        np.asarray(inp["jaw_params"], f32).reshape(B, 3),
        np.asarray(inp["eye_pose"], f32).reshape(B, 6),
        lh[:, lvl[0:5]].reshape(B, 15), rh[:, lvl[0:5]].reshape(B, 15),
        lh[:, lvl[5:10]].reshape(B, 15), rh[:, lvl[5:10]].reshape(B, 15),
        lh[:, lvl[10:15]].reshape(B, 15), rh[:, lvl[10:15]].reshape(B, 15),
    ], axis=1)                                               # [B,165]

    ep = np.asarray(inp["eyelid_params"], f32)
    aux = np.concatenate([
        np.asarray(inp["head_scale"], f32)[:, None],
        np.asarray(inp["left_hand_scale"], f32)[:, None],
        np.asarray(inp["right_hand_scale"], f32)[:, None],
        ep[:, 0:1], ep[:, 1:2],
        np.asarray(inp["head_pos_offset"], f32),
        np.asarray(inp["left_hand_pos_offset"], f32),
        np.asarray(inp["right_hand_pos_offset"], f32),
    ], axis=1)                                               # [128, 14]

    def beta_T(second):
        b = np.concatenate([np.asarray(inp["shape_params"], f32), second], 1)
        bt = np.zeros((384, B), f32)
        bt[:NL] = b.T
        bt[NL] = 1.0
        return bt.reshape(3, 128, B)

    betaT_s = beta_T(np.asarray(inp["body_exp"], f32))
    betaT_f = beta_T(np.asarray(inp["flame_exp"], f32))

    joff = np.asarray(inp["joints_offset"], f32)

    # ---- J regression folded into weights: Jmat = Jreg @ [shapedirs | tmpl] ----
    def jdirs(jr, sd, tmpl, nj):
        ext = np.concatenate([sd, tmpl[:, :, None]], axis=2)      # [V,3,L+1]
        jm = jr @ ext.reshape(ext.shape[0], -1)                   # [nj, 3*(L+1)]
        return jm.reshape(nj, 3, ext.shape[2])

    jm_s = jdirs(jr_s_np, sd_s_np, tmpl_s, 55)                    # [55,3,351]
    jm_f = jdirs(jr_f_np, sd_f_np, tmpl_f, 5)                     # [5,3,351]
    jm_m = jdirs(jr_m_np, sd_m_np, tmpl_m, 16)                    # [16,3,11]

    # device lhsT layout: jd[lk, l, c*nj + j] = jm[j, c, lk*128 + l]
    def jd_pack(jm, nj):
        out = np.zeros((3, 128, 3 * nj), f32)
        L = jm.shape[2]
        for lk in range(3):
            l0, l1 = lk * 128, min((lk + 1) * 128, L)
            blk = jm[:, :, l0:l1]                                 # [nj,3,n]
            out[lk, :l1 - l0] = blk.transpose(2, 1, 0).reshape(l1 - l0, 3 * nj)
        return out

    jd_s = jd_pack(jm_s, 55)
    jd_f = jd_pack(jm_f, 5)

    # MANO joints are batch-independent (shared betas): compute on host.
    bm_ext = np.concatenate([np.asarray(inp["mano_betas"], f32)[0], [1.0]])
    jmano = np.einsum('jcl,l->jc', jm_m, bm_ext)                  # [16,3]
    # level-major J for hands (same for L and R), c-major: [3,15] -> 45
    jm_lvl = jmano[HAND_LVL]                                      # [15,3]
    jmb = np.broadcast_to(jm_lvl.T.reshape(1, 45), (B, 45)).copy()
    jm0 = np.broadcast_to(jmano[0][None], (B, 3)).copy()

    betam = np.zeros((11, 1), f32)
    betam[:10, 0] = np.asarray(inp["mano_betas"], f32)[0]
    betam[10, 0] = 1.0

    # joints_offset in batch layout
    joffb = np.ascontiguousarray(joff.transpose(0, 2, 1)).reshape(B, 165)

    # pack all small constants: cb_p (pose; tiny, lands first) + cb_c
    cb_p = np.zeros((128, 179), f32)
    cb_p[:, 0:165] = aa
    cb_p[:, 165:179] = aux
    cb_c = np.zeros((128, 341), f32)
    cb_c[:, 0:128] = np.eye(128, dtype=f32)
    cb_c[:, 128:293] = joffb
    cb_c[:, 293:338] = jmb
    cb_c[:, 338:341] = jm0
    cb_b = np.zeros((128, 1309), f32)
    cb_b[:, 0:384] = betaT_s.transpose(1, 0, 2).reshape(128, 384)
    cb_b[:, 384:768] = betaT_f.transpose(1, 0, 2).reshape(128, 384)
    cb_b[:, 768:1263] = jd_s.transpose(1, 0, 2).reshape(128, 495)
    cb_b[:, 1263:1308] = jd_f.transpose(1, 0, 2).reshape(128, 45)
    cb_b[0:11, 1308] = betam[:, 0]
    rep = dict(cb_p=cb_p, cb_c=cb_c, cb_b=cb_b)

    # mano posedirs rows permuted to level-major order
    pd_m_perm = np.empty_like(pd_m_np)
    for pos, j in enumerate(HAND_LVL):
        pd_m_perm[pos * 9:(pos + 1) * 9] = pd_m_np[(j - 1) * 9:j * 9]

    in_maps = []
    vid_all = np.full((NCORES, ROWS), -1, np.int64)

    for c in range(NCORES):
        p_ids = plain_sv[pl_sp[c]:pl_sp[c + 1]]
        h_pos = np.arange(hd_sp[c], hd_sp[c + 1])
        h_sv, h_fv = head_sv[h_pos], head_ix[h_pos]
        l_pos = np.arange(hl_sp[c], hl_sp[c + 1])
        r_pos = l_pos
        l_sv, r_sv = s2l[l_pos], s2r[r_pos]

        vid = np.full(ROWS, -1, np.int64)
        vid[:len(p_ids)] = p_ids
        vid[N_PLAIN:N_PLAIN + len(h_sv)] = h_sv
        vid[N_PLAIN + N_HEAD:N_PLAIN + N_HEAD + len(l_sv)] = l_sv
        vid[N_PLAIN + N_HEAD + N_HL:N_PLAIN + N_HEAD + N_HL + len(r_sv)] = r_sv
        vid_all[c] = vid
        vok = vid >= 0
        vc = np.where(vok, vid, 0)

        # smplx shapedirs slab, PLAIN chunks only: [6, 128(p=l), (c, lk, v)]
        pvc = vc[:N_PLAIN]
        pvok = vok[:N_PLAIN]
        sdp = np.zeros((N_PLAIN, 3, 384), f32)
        sdp[:, :, :NL] = np.where(pvok[:, None, None], sd_s_np[pvc], 0.0)
        sdp[:, :, NL] = np.where(pvok[:, None], tmpl_s[pvc], 0.0)
        slab = sdp.reshape(NCH_PLAIN, 128, 3, 3, 128).transpose(0, 4, 2, 3, 1)
        sd_s = np.ascontiguousarray(slab).reshape(NCH_PLAIN, 128, 1152)

        # smplx posedirs, all chunks
        colv = vc[:, None] * 3 + np.arange(3)[None, :]
        pdv = pd_s_np[:PD_S_K][:, colv]
        pdv = np.where(vok[None, :, None], pdv, 0.0)
        pdv = pdv.reshape(PD_S_K, NCH, 128, 3).transpose(1, 0, 3, 2)
        pd_s_a = np.ascontiguousarray(pdv[:, :128]).reshape(NCH, 128, 384)
        pd_s_b = np.ascontiguousarray(pdv[:, 128:]).reshape(NCH, PD_S_K - 128, 384)

        # folded smplx weights stationary [22, NCH*128]
        w_s = np.ascontiguousarray(
            np.where(vok[:, None], w_fold[vc], 0.0)
            .reshape(NCH, 128, 22).transpose(0, 2, 1))

        # flame: 3 gathered head chunks only
        fg = _pad_ids(h_fv, N_HEAD)
        fok = fg >= 0
        fc = np.where(fok, fg, 0)
        sdfp = np.zeros((N_HEAD, 3, 384), f32)
        sdfp[:, :, :NL] = np.where(fok[:, None, None], sd_f_np[fc], 0.0)
        sdfp[:, :, NL] = np.where(fok[:, None], tmpl_f[fc], 0.0)
        slab = sdfp.reshape(3, 128, 3, 3, 128).transpose(0, 4, 2, 3, 1)
        sd_f = np.ascontiguousarray(slab).reshape(3, 128, 1152)

        colf = fc[:, None] * 3 + np.arange(3)[None, :]
        pdfv = pd_f_np[9:36][:, colf]
        pdfv = np.where(fok[None, :, None], pdfv, 0.0)
        pdfv = pdfv.reshape(PD_F_K, 3, 128, 3).transpose(1, 0, 3, 2)
        pd_f = np.ascontiguousarray(pdfv).reshape(3, PD_F_K, 384)

        # flame stationary [64, 3*128]: jaw/eyes, id row, eyelid rows
        w_f = np.zeros((3, NJ64, 128), f32)
        for k in range(3):
            rows, ok = fc[k * 128:(k + 1) * 128], fok[k * 128:(k + 1) * 128]
            w_f[k, 22:25] = np.where(ok[None, :], w_f_np[rows][:, 2:5].T, 0.0)
            w_f[k, 55] = np.where(ok, w_f_np[rows][:, 0] + w_f_np[rows][:, 1], 0.0)
            w_f[k, 58:61] = np.where(ok[None, :], re_np[rows].T, 0.0)
            w_f[k, 61:64] = np.where(ok[None, :], le_np[rows].T, 0.0)

        # mano hands
        m_rows = np.stack([_pad_ids(l_pos, 128), _pad_ids(r_pos, 128)])
        mok = m_rows >= 0
        mc = np.where(mok, m_rows, 0)
        sd_m = np.zeros((2, 11, 384), f32)
        pd_m_a = np.zeros((2, 128, 384), f32)
        pd_m_b = np.zeros((2, PD_M_K - 128, 384), f32)
        w_m = np.zeros((2, NJ64, 128), f32)
        for h in range(2):
            sdm = np.where(mok[h][:, None, None], sd_m_np[mc[h]], 0.0)
            sd_m[h, :10] = sdm.transpose(2, 1, 0).reshape(10, 384)
            sd_m[h, 10] = np.where(mok[h][:, None], tmpl_m[mc[h]], 0.0).T.reshape(384)
            colm = mc[h][:, None] * 3 + np.arange(3)[None, :]
            pdm = pd_m_perm[:, colm]
            pdm = np.where(mok[h][None, :, None], pdm, 0.0).transpose(0, 2, 1)
            pd_m_a[h] = pdm[:128].reshape(128, 384)
            pd_m_b[h] = pdm[128:].reshape(PD_M_K - 128, 384)
            wm = np.where(mok[h][None, :], w_m_np[mc[h]].T, 0.0)  # [16,128]
            # posed rows: level-major, interleaved L/R blocks
            for lv in range(3):
                r0 = 25 + lv * 10 + h * 5
                w_m[h, r0:r0 + 5] = wm[HAND_LVL[lv * 5:(lv + 1) * 5]]
            w_m[h, 56 + h] = wm[0]

        m = dict(rep)
        pk = lambda a: np.ascontiguousarray(a.transpose(1, 0, 2)).reshape(a.shape[1], -1)
        m.update(sd_s0=sd_s[0:2].transpose(1, 0, 2).reshape(128, -1),
                 sd_s1=sd_s[2:4].transpose(1, 0, 2).reshape(128, -1),
                 sd_s2=sd_s[4:6].transpose(1, 0, 2).reshape(128, -1),
                 pd_s_a=pk(pd_s_a), pd_s_b=pk(pd_s_b), w_s=pk(w_s),
                 sd_f=pk(sd_f), pd_f=pk(pd_f), w_f=pk(w_f),
                 sd_m=pk(sd_m), pd_m_a=pk(pd_m_a), pd_m_b=pk(pd_m_b), w_m=pk(w_m))
        out = {}
        for k, v in m.items():
            v = np.ascontiguousarray(v)
            if k in BF16_INPUTS:
                out[k] = np.ascontiguousarray(v.astype(BF16NP))
            elif k in F16_INPUTS:
                out[k] = np.ascontiguousarray(v.astype(F16NP))
            else:
                out[k] = np.ascontiguousarray(v, f32)
        in_maps.append(out)

    return in_maps, vid_all


# ================================================================ device IR

def _build_nc():
    nc = bacc.Bacc("TRN2", target_bir_lowering=False, debug=False,
                   num_devices=NCORES)
    di = {}

    def din(name, shape):
        dt = BF16 if name in BF16_INPUTS else (F16 if name in F16_INPUTS else F32)
        di[name] = nc.dram_tensor(name, list(shape), dt, kind="ExternalInput").ap()

    din("cb_p", (128, 179)); din("cb_c", (128, 341))
    din("cb_b", (128, 1309))
    din("sd_s0", (128, 2304)); din("sd_s1", (128, 2304)); din("sd_s2", (128, 2304))
    din("pd_s_a", (128, NCH * 384)); din("pd_s_b", (PD_S_K - 128, NCH * 384))
    din("w_s", (22, NCH * 128))
    din("sd_f", (128, 3 * 1152)); din("pd_f", (PD_F_K, 3 * 384))
    din("w_f", (NJ64, 3 * 128))
    din("sd_m", (11, 2 * 384)); din("pd_m_a", (128, 2 * 384))
    din("pd_m_b", (PD_M_K - 128, 2 * 384)); din("w_m", (NJ64, 2 * 128))

    out_d = nc.dram_tensor("out", [ROWS, 384], F16, kind="ExternalOutput").ap()
    dbg_d = None
    if DEBUG:
        dbg_d = nc.dram_tensor("dbg", [128, 2560], F32, kind="ExternalOutput").ap()

    with tile.TileContext(nc) as tc:
        _emit(nc, tc, di, out_d, dbg_d)
    nc.compile()
    return nc


def _emit(nc, tc, di, out_d, dbg_d=None):
    es = ExitStack()
    persist = es.enter_context(tc.tile_pool(name="persist", bufs=1))
    slabs = es.enter_context(tc.tile_pool(name="slabs", bufs=3))
    psum = es.enter_context(tc.tile_pool(name="psum", bufs=1, space="PSUM"))

    V, S, G, T, DMA = nc.vector, nc.scalar, nc.gpsimd, nc.tensor, nc.sync
    P = nc.gpsimd

    def ptile(shape, name, dt=F32):
        return persist.tile(list(shape), dt, tag=name, name=name)

    # ---------------- DMAs: constants first, then streaming slabs ---------
    cb_p = ptile((128, 179), "cb_p")
    DMA.dma_start(cb_p[:], di["cb_p"][:])
    cb_b = ptile((128, 1309), "cb_b", BF16)
    DMA.dma_start(cb_b[:], di["cb_b"][:])
    cb_c = ptile((128, 341), "cb_c")
    DMA.dma_start(cb_c[:], di["cb_c"][:])

    aa = cb_p[:][:, 0:165]
    aux = cb_p[:][:, 165:179]
    c32 = cb_c[:]
    ident = c32[:, 0:128]
    joffb = c32[:, 128:293]
    jmb = c32[:, 293:338]       # [B, (c,15)] level-major hand J (L==R)
    jm0 = c32[:, 338:341]
    c16 = cb_b[:]
    betaT_s = c16[:, 0:384]
    betaT_f = c16[:, 384:768]
    jd_s = c16[:, 768:1263]
    jd_f = c16[:, 1263:1308]
    betam = cb_b[0:11, 1308:1309]

    # activation-table preload: two dummy activations on Scalar before its
    # DMA issues, so sqrt/sin tables are resident before rodrigues needs them
    dum = ptile((128, 4), "dum")
    zero_t = ptile((B, 1), "rg_zero")
    G.memset(zero_t[:], 0.0)
    G.memset(dum[:], 0.0)
    S.activation(dum[:, 0:2], dum[:, 2:4], AF.Sin, bias=zero_t[:])
    S.activation(dum[:, 0:2], dum[:, 2:4], AF.Sqrt, bias=zero_t[:])

    # stage-A-critical inputs on the sync queue, in consumption order
    sd_s_t = [ptile((128, 2304), f"sd_s{i}", BF16) for i in range(3)]
    pd_a_t = ptile((128, NCH * 384), "pd_a", BF16)
    pd_b_t = ptile((PD_S_K - 128, NCH * 384), "pd_b", BF16)
    DMA.dma_start(sd_s_t[0][:], di["sd_s0"][:])
    DMA.dma_start(pd_a_t[:, 0:3 * 384], di["pd_s_a"][:, 0:3 * 384])
    DMA.dma_start(pd_b_t[:, 0:3 * 384], di["pd_s_b"][:, 0:3 * 384])
    DMA.dma_start(sd_s_t[1][:], di["sd_s1"][:])
    DMA.dma_start(pd_a_t[:, 3 * 384:6 * 384], di["pd_s_a"][:, 3 * 384:6 * 384])
    DMA.dma_start(pd_b_t[:, 3 * 384:6 * 384], di["pd_s_b"][:, 3 * 384:6 * 384])
    DMA.dma_start(sd_s_t[2][:], di["sd_s2"][:])
    sd_f_t = ptile((128, 3 * 1152), "sd_f", BF16)
    DMA.dma_start(sd_f_t[:], di["sd_f"][:])
    DMA.dma_start(pd_a_t[:, 6 * 384:], di["pd_s_a"][:, 6 * 384:])
    DMA.dma_start(pd_b_t[:, 6 * 384:], di["pd_s_b"][:, 6 * 384:])
    pd_f_t = ptile((PD_F_K, 3 * 384), "pd_f", BF16)
    DMA.dma_start(pd_f_t[:], di["pd_f"][:])

    w_s_t = ptile((22, NCH * 128), "w_s", F16)
    DMA.dma_start(w_s_t[:], di["w_s"][:])
    sd_m_t = ptile((11, 768), "sd_m", BF16)
    DMA.dma_start(sd_m_t[:], di["sd_m"][:])
    pd_ma_t = ptile((128, 768), "pd_ma", BF16)
    DMA.dma_start(pd_ma_t[:], di["pd_m_a"][:])
    pd_mb_t = ptile((PD_M_K - 128, 768), "pd_mb", BF16)
    DMA.dma_start(pd_mb_t[:], di["pd_m_b"][:])
    w_f_t = ptile((NJ64, 384), "w_f", F16)
    DMA.dma_start(w_f_t[:], di["w_f"][:])
    w_m_t = ptile((NJ64, 256), "w_m", F16)
    DMA.dma_start(w_m_t[:], di["w_m"][:])

    # ---------------- joints (6 small MMs, one accumulation group/bank) ---
    jp = psum.tile([128, 512], F32, tag="ps", bufs=2)
    for lk in range(3):
        T.matmul(jp[:, 0:165], betaT_s[:, lk * 128:(lk + 1) * 128],
                 jd_s[:, lk * 165:(lk + 1) * 165],
                 start=(lk == 0), stop=(lk == 2))
    jpf = psum.tile([128, 512], F32, tag="ps", bufs=2)
    for lk in range(3):
        T.matmul(jpf[:, 0:15], betaT_f[:, lk * 128:(lk + 1) * 128],
                 jd_f[:, lk * 15:(lk + 1) * 15],
                 start=(lk == 0), stop=(lk == 2))

    jb = ptile((B, 165), "jb")
    jfb = ptile((B, 15), "jfb")
    V.tensor_add(jb[:], jp[:, 0:165], joffb)
    S.copy(jfb[:], jpf[:, 0:15])

    # ---------------- rodrigues (55 rots, v3 order) -----------------------
    rot = ptile((B, 55 * 9), "rot")
    _rodrigues(nc, aa, rot, ptile, zero_t)
    rot4 = rot[:].rearrange("p (j x) -> p j x", x=9)
    rot5 = rot[:].rearrange("p (j m n) -> p j m n", m=3, n=3)

    # ---------------- pose features + transposes --------------------------
    def pf_diag_sub(t9, n):
        V.tensor_scalar_add(t9[:, :, 0:9:4], t9[:, :, 0:9:4], -1.0)

    pf_s = ptile((B, 189), "pf_s")
    t9 = pf_s[:].rearrange("p (j x) -> p j x", x=9)
    V.tensor_copy(t9, rot4[:, 1:22, :])
    pf_diag_sub(t9, 21)
    pf_f = ptile((B, 27), "pf_f")
    t9 = pf_f[:].rearrange("p (j x) -> p j x", x=9)
    V.tensor_copy(t9, rot4[:, 22:25, :])
    pf_diag_sub(t9, 3)
    # hands: L blocks at lvl*10+0, R at lvl*10+5 (within rows 25:55)
    pf_m = [ptile((B, 135), f"pf_m{h}") for h in range(2)]
    handrot = rot[:, 25 * 9:55 * 9].rearrange("p (l h x) -> p l h x", l=3, h=2)
    for h in range(2):
        t9 = pf_m[h][:].rearrange("p (l x) -> p l x", l=3)
        P.tensor_copy(t9, handrot[:, :, h, :])
        P.tensor_scalar_add(
            pf_m[h][:].rearrange("p (j x) -> p j x", x=9)[:, :, 0:9:4],
            pf_m[h][:].rearrange("p (j x) -> p j x", x=9)[:, :, 0:9:4], -1.0)

    def transpose_to(dst_ap, src_ap):
        pp = psum.tile([128, 512], F32, tag="ps", bufs=2)
        k, n = src_ap.shape[0], src_ap.shape[1]
        T.matmul(pp[:n, :k], src_ap, ident[0:k, 0:k], is_transpose=True,
                 start=True, stop=True)
        S.copy(dst_ap, pp[:n, :k])

    pfT_s_a = ptile((128, 128), "pfT_s_a", BF16)
    pfT_s_b = ptile((PD_S_K - 128, 128), "pfT_s_b", BF16)
    transpose_to(pfT_s_a[:], pf_s[:, 0:128])
    transpose_to(pfT_s_b[:], pf_s[:, 128:PD_S_K])
    pfT_f = ptile((PD_F_K, 128), "pfT_f", BF16)
    transpose_to(pfT_f[:], pf_f[:, :])
    pfT_m_a = [ptile((128, 128), f"pfT_m{h}a", BF16) for h in range(2)]
    pfT_m_b = [ptile((PD_M_K - 128, 128), f"pfT_m{h}b", BF16) for h in range(2)]

    epp = ptile((B, 2), "epp")
    negls = ptile((B, 1), "negls")
    P.tensor_scalar_mul(negls[:], aux[:, 1:2], -1.0)

    # ---------------- M matrices: [R | (I-R) J] ---------------------------
    Mb = ptile((B, 55 * 12), "Mb")
    M4 = Mb[:].rearrange("p (j m n) -> p j m n", m=3, n=4)
    Ab = ptile((B, NJ64 * 12), "Ab")
    A4 = Ab[:].rearrange("p (j m n) -> p j m n", m=3, n=4)
    scr = ptile((B, 264), "scr")    # V-side fk/M scratch
    scrp = ptile((B, 264), "scrp")  # P-side fk/M scratch

    # rotation part (one big copy)
    V.tensor_copy(M4[:, 0:55, :, 0:3], rot5[:, 0:55])

    # translation: t = J - R J.  Jfull is [B, (c, nJ)] c-major; section
    # joint i corresponds to J column c*nJ + joff + i.
    def m_trans(eng, sc_t, j0, nj, Jfull, nJ, joff):
        dst = M4[:, j0:j0 + nj, :, 3]
        sc3 = sc_t[:].rearrange("p (j m) -> p j m", m=3)[:, 0:nj]
        for k in range(3):
            jk = Jfull[:, k * nJ + joff:k * nJ + joff + nj]
            jk = jk.unsqueeze(2).broadcast_to([B, nj, 3])
            rk = rot5[:, j0:j0 + nj, :, k]
            if k == 0:
                eng.tensor_mul(dst, rk, jk)
            else:
                eng.tensor_mul(sc3, rk, jk)
                eng.tensor_add(dst, dst, sc3)
        # dst = J^T(m-major view) - dst
        jm_ = Jfull.rearrange("p (c j) -> p j c", c=3)[:, joff:joff + nj]
        eng.tensor_sub(dst, jm_, dst)

    m_trans(V, scr, 0, 22, jb[:], 55, 0)
    m_trans(V, scr, 22, 3, jfb[:], 5, 2)
    # hands: J = jmb (c-major 15, level-major), shared by L and R.
    # M trans rows 25:55 viewed as [B, lvl(3), hj(10), x(12)]; L = hj 0:5.
    hx = Mb[:, 25 * 12:55 * 12].rearrange("p (l hj x) -> p l hj x", hj=10, x=12)
    hrot = rot[:, 25 * 9:55 * 9].rearrange("p (l hj x) -> p l hj x", hj=10, x=9)
    scp4 = scrp[:, 0:180].rearrange("p (l hj m) -> p l hj m", l=3, m=3)
    jmTv = jmb[:].rearrange("p (m l j) -> p l j m", m=3, l=3)    # [B,3,5,3]
    for h in range(2):
        dsth = hx[:, :, h * 5:(h + 1) * 5, 3:12:4]               # [B,3,5,3]
        sch = scp4[:, :, h * 5:(h + 1) * 5, :]
        for k in range(3):
            jsl = jmb[:, k * 15:(k + 1) * 15].rearrange("p (l j) -> p l j", l=3)
            jsl = jsl.unsqueeze(3).broadcast_to([B, 3, 5, 3])
            rk = hrot[:, :, h * 5:(h + 1) * 5, k:9:3]            # [B,3,5,3] m
            if k == 0:
                P.tensor_mul(dsth, rk, jsl)
            else:
                P.tensor_mul(sch, rk, jsl)
                P.tensor_add(dsth, dsth, sch)
        P.tensor_sub(dsth, jmTv, dsth)

    # ---------------- FK: A = A_par o M -----------------------------------
    G.memset(Ab[:, 55 * 12:NJ64 * 12], 0.0)

    # direct rows: body root, flame jaw/eyes, hands d1 (25:35)
    V.tensor_copy(A4[:, 0:1], M4[:, 0:1])
    V.tensor_copy(A4[:, 22:25], M4[:, 22:25])
    P.tensor_copy(A4[:, 25:35], M4[:, 25:35])

    def compose(eng, dst, par, dT, n, scr_t):
        # dst = par o dT ; par [B,n,3,4] (world), dT [B,n,3,4] (rel)
        sc = scr_t[:].rearrange("p (j m n) -> p j m n", m=3, n=4)[:, :n]
        for k in range(3):
            a_k = par[:, :, :, k:k + 1].broadcast_to([B, n, 3, 4])
            t_k = dT[:, :, k:k + 1, :].broadcast_to([B, n, 3, 4])
            if k == 0:
                eng.tensor_mul(dst, a_k, t_k)
            else:
                eng.tensor_mul(sc, a_k, t_k)
                eng.tensor_add(dst, dst, sc)
        eng.tensor_add(dst[:, :, :, 3], dst[:, :, :, 3], par[:, :, :, 3])

    # pair products on P (independent of trunk): X16,17 = M13,14 o M16,17 ;
    # X20,21 = M18,19 o M20,21
    Xb = ptile((B, 4 * 12), "Xb")
    X4 = Xb[:].rearrange("p (j m n) -> p j m n", m=3, n=4)
    compose(P, X4[:, 0:2], M4[:, 13:15], M4[:, 16:18], 2, scrp)
    compose(P, X4[:, 2:4], M4[:, 18:20], M4[:, 20:22], 2, scrp)
    # hand levels on P: d2 (35:45 <- 25:35), d3 (45:55 <- 35:45)
    compose(P, A4[:, 35:45], A4[:, 25:35], M4[:, 35:45], 10, scrp)
    compose(P, A4[:, 45:55], A4[:, 35:45], M4[:, 45:55], 10, scrp)

    # body trunk on V; arm chain A16,17 = A9 o X1 and A20,21 = A16,17 o X2
    # runs on P so V and P advance in parallel after A9.
    compose(V, A4[:, 1:4], A4[:, 0:1].broadcast_to([B, 3, 3, 4]),
            M4[:, 1:4], 3, scr)
    compose(V, A4[:, 4:7], A4[:, 1:4], M4[:, 4:7], 3, scr)
    compose(V, A4[:, 7:10], A4[:, 4:7], M4[:, 7:10], 3, scr)
    compose(P, A4[:, 16:18], A4[:, 9:10].broadcast_to([B, 2, 3, 4]),
            X4[:, 0:2], 2, scrp)
    compose(P, A4[:, 20:22], A4[:, 16:18], X4[:, 2:4], 2, scrp)
    compose(V, A4[:, 10:13], A4[:, 7:10], M4[:, 10:13], 3, scr)
    compose(V, A4[:, 13:15], A4[:, 9:10].broadcast_to([B, 2, 3, 4]),
            M4[:, 13:15], 2, scr)
    compose(V, A4[:, 15:16], A4[:, 12:13], M4[:, 15:16], 1, scr)
    compose(V, A4[:, 18:20], A4[:, 16:18], M4[:, 18:20], 2, scr)

    # ---------------- biases (need only jb/jfb, run during FK tail) -------
    jb3 = jb[:].rearrange("p (c j) -> p c j", c=3)
    jf3 = jfb[:].rearrange("p (c j) -> p c j", c=3)
    bias9 = ptile((B, 9), "bias9")      # 0:3 flame, 3:6 L, 6:9 R
    hm = ptile((B, 8), "hm")
    V.tensor_add(hm[:, 0:3], jb3[:, :, 23], jb3[:, :, 24])
    V.tensor_add(hm[:, 3:6], jf3[:, :, 3], jf3[:, :, 4])
    V.tensor_sub(hm[:, 0:3], hm[:, 0:3], hm[:, 3:6])
    V.tensor_scalar_mul(hm[:, 0:3], hm[:, 0:3], 0.5)
    V.tensor_add(bias9[:, 0:3], hm[:, 0:3], aux[:, 5:8])
    # bias_l = (lpo - jm0)*[-1,1,1] + tbj20 ; bias_r = rpo - jm0 + tbj21
    V.tensor_sub(hm[:, 3:6], aux[:, 8:11], jm0)
    V.tensor_sub(bias9[:, 3:4], jb3[:, 0:1, 20], hm[:, 3:4])
    V.tensor_add(bias9[:, 4:6], hm[:, 4:6], jb3[:, 1:3, 20])
    V.tensor_sub(hm[:, 3:6], aux[:, 11:14], jm0)
    V.tensor_add(bias9[:, 6:9], hm[:, 3:6], jb3[:, :, 21])
    V.tensor_mul(epp[:], aux[:, 3:5], aux[:, 0:1].broadcast_to([B, 2]))

    # ---------------- folds: scale/mirror/bias + virtual rows -------------
    # flame rows 22:25: A = s*A ; trans += bias_f
    V.tensor_scalar_mul(Ab[:, 22 * 12:25 * 12], Ab[:, 22 * 12:25 * 12],
                        aux[:, 0:1])
    V.tensor_add(A4[:, 22:25, :, 3], A4[:, 22:25, :, 3],
                 bias9[:, 0:3].unsqueeze(1).broadcast_to([B, 3, 3]))
    # hands: rows 25:55 as [B, lvl(3), hj(10), x(12)]; L = hj 0:5, R = 5:10
    hAx = Ab[:, 25 * 12:55 * 12].rearrange("p (l hj x) -> p l hj x",
                                           hj=10, x=12)
    # L: row m=0 (x 0:4) *= -s_l ; m=1,2 (x 4:12) *= s_l ; R: all *= s_r
    P.tensor_mul(hAx[:, :, 0:5, 0:4], hAx[:, :, 0:5, 0:4],
                 negls[:, 0:1].unsqueeze(2).unsqueeze(3)
                 .broadcast_to([B, 3, 5, 4]))
    P.tensor_mul(hAx[:, :, 0:5, 4:12], hAx[:, :, 0:5, 4:12],
                 aux[:, 1:2].unsqueeze(2).unsqueeze(3)
                 .broadcast_to([B, 3, 5, 8]))
    P.tensor_mul(hAx[:, :, 5:10, 0:12], hAx[:, :, 5:10, 0:12],
                 aux[:, 2:3].unsqueeze(2).unsqueeze(3)
                 .broadcast_to([B, 3, 5, 12]))
    # trans += bias  (x = 3,7,11 gives m = 0,1,2)
    P.tensor_add(hAx[:, :, 0:5, 3:12:4], hAx[:, :, 0:5, 3:12:4],
                 bias9[:, 3:6].unsqueeze(1).unsqueeze(2)
                 .broadcast_to([B, 3, 5, 3]))
    P.tensor_add(hAx[:, :, 5:10, 3:12:4], hAx[:, :, 5:10, 3:12:4],
                 bias9[:, 6:9].unsqueeze(1).unsqueeze(2)
                 .broadcast_to([B, 3, 5, 3]))
    # id rows 55,56,57: diag scale + bias trans
    V.tensor_copy(Ab[:, 55 * 12:55 * 12 + 11:5],
                  aux[:, 0:1].broadcast_to([B, 3]))
    V.tensor_copy(A4[:, 55, :, 3], bias9[:, 0:3])
    P.tensor_copy(Ab[:, 56 * 12:56 * 12 + 1], negls[:])
    P.tensor_copy(Ab[:, 56 * 12 + 5:56 * 12 + 11:5],
                  aux[:, 1:2].broadcast_to([B, 2]))
    P.tensor_copy(A4[:, 56, :, 3], bias9[:, 3:6])
    P.tensor_copy(Ab[:, 57 * 12:57 * 12 + 11:5],
                  aux[:, 2:3].broadcast_to([B, 3]))
    P.tensor_copy(A4[:, 57, :, 3], bias9[:, 6:9])
    # eyelid rows: trans diag = epp
    V.tensor_copy(Ab[:, 58 * 12 + 3:58 * 12 + 36:16],
                  epp[:, 1:2].broadcast_to([B, 3]))
    V.tensor_copy(Ab[:, 61 * 12 + 3:61 * 12 + 36:16],
                  epp[:, 0:1].broadcast_to([B, 3]))

    # ---------------- stage A: blend shapes (fp16 v_posed out) ------------
    vp16 = [ptile((128, 384), f"vp{i}", F16) for i in range(NCH)]
    vpf16 = [ptile((128, 384), f"vpf{h}", F16) for h in range(3)]
    vpm16 = [ptile((128, 384), f"vpm{h}", F16) for h in range(2)]

    def stage_a_chunk(i):
        # NB: per PSUM bank only one accumulation group may be open at a
        # time, so each (chunk, c3) group runs start..stop contiguously.
        pq = psum.tile([128, 512], F32, tag="ps", bufs=2)
        pda = pd_a_t[:, i * 384:(i + 1) * 384]
        pdb = pd_b_t[:, i * 384:(i + 1) * 384]
        if i in CH_PLAIN:
            sdt = sd_s_t[i // 2][:, (i % 2) * 1152:(i % 2 + 1) * 1152]
            for c3 in range(3):
                for lk in range(3):
                    T.matmul(pq[:, c3 * 128:(c3 + 1) * 128],
                             sdt[:, (c3 * 3 + lk) * 128:(c3 * 3 + lk + 1) * 128],
                             betaT_s[:, lk * 128:(lk + 1) * 128],
                             start=(lk == 0), stop=False)
                T.matmul(pq[:, c3 * 128:(c3 + 1) * 128],
                         pda[:, c3 * 128:(c3 + 1) * 128], pfT_s_a[:],
                         start=False, stop=False)
                T.matmul(pq[:, c3 * 128:(c3 + 1) * 128],
                         pdb[:, c3 * 128:(c3 + 1) * 128], pfT_s_b[:],
                         start=False, stop=True)
        else:
            for c3 in range(3):
                T.matmul(pq[:, c3 * 128:(c3 + 1) * 128],
                         pda[:, c3 * 128:(c3 + 1) * 128], pfT_s_a[:],
                         start=True, stop=False)
                T.matmul(pq[:, c3 * 128:(c3 + 1) * 128],
                         pdb[:, c3 * 128:(c3 + 1) * 128], pfT_s_b[:],
                         start=False, stop=True)
        S.copy(vp16[i][:], pq[:, 0:384])

    def stage_a_flame(h):
        sdt = sd_f_t[:, h * 1152:(h + 1) * 1152]
        pdf = pd_f_t[:, h * 384:(h + 1) * 384]
        pq = psum.tile([128, 512], F32, tag="ps", bufs=2)
        for c3 in range(3):
            for lk in range(3):
                T.matmul(pq[:, c3 * 128:(c3 + 1) * 128],
                         sdt[:, (c3 * 3 + lk) * 128:(c3 * 3 + lk + 1) * 128],
                         betaT_f[:, lk * 128:(lk + 1) * 128],
                         start=(lk == 0), stop=False)
            T.matmul(pq[:, c3 * 128:(c3 + 1) * 128],
                     pdf[:, c3 * 128:(c3 + 1) * 128], pfT_f[:],
                     start=False, stop=True)
        S.copy(vpf16[h][:], pq[:, 0:384])

    def stage_a_mano(h):
        sdt = sd_m_t[:, h * 384:(h + 1) * 384]
        pq = psum.tile([128, 512], F32, tag="ps", bufs=2)
        for c3 in range(3):
            T.matmul(pq[:, 384 + c3:385 + c3], sdt[:, c3 * 128:(c3 + 1) * 128],
                     betam, start=True, stop=True)
        pda = pd_ma_t[:, h * 384:(h + 1) * 384]
        pdb = pd_mb_t[:, h * 384:(h + 1) * 384]
        for c3 in range(3):
            T.matmul(pq[:, c3 * 128:(c3 + 1) * 128],
                     pda[:, c3 * 128:(c3 + 1) * 128], pfT_m_a[h][:],
                     start=True, stop=False)
            T.matmul(pq[:, c3 * 128:(c3 + 1) * 128],
                     pdb[:, c3 * 128:(c3 + 1) * 128], pfT_m_b[h][:],
                     start=False, stop=True)
        vshm = ptile((128, 3), f"vshm{h}")
        S.copy(vshm[:], pq[:, 384:387])
        for c3 in range(3):
            S.add(vpm16[h][:, c3 * 128:(c3 + 1) * 128],
                  pq[:, c3 * 128:(c3 + 1) * 128], vshm[:, c3:c3 + 1])

    for i in range(NCH_PLAIN - 1):
        stage_a_chunk(i)

    # ---------------- stage A: blend shapes (fp16 v_posed out) ------------
    vp16 = [ptile((128, 384), f"vp{i}", F16) for i in range(NCH)]
    vpf16 = [ptile((128, 384), f"vpf{h}", F16) for h in range(3)]
    vpm16 = [ptile((128, 384), f"vpm{h}", F16) for h in range(2)]

    def stage_a_chunk(i):
        # NB: per PSUM bank only one accumulation group may be open at a
        # time, so each (chunk, c3) group runs start..stop contiguously.
        pq = psum.tile([128, 512], F32, tag="ps", bufs=2)
        pda = pd_a_t[:, i * 384:(i + 1) * 384]
        pdb = pd_b_t[:, i * 384:(i + 1) * 384]
        if i in CH_PLAIN:
            sdt = sd_s_t[i // 2][:, (i % 2) * 1152:(i % 2 + 1) * 1152]
            for c3 in range(3):
                for lk in range(3):
                    T.matmul(pq[:, c3 * 128:(c3 + 1) * 128],
                             sdt[:, (c3 * 3 + lk) * 128:(c3 * 3 + lk + 1) * 128],
                             betaT_s[:, lk * 128:(lk + 1) * 128],
                             start=(lk == 0), stop=False)
                T.matmul(pq[:, c3 * 128:(c3 + 1) * 128],
                         pda[:, c3 * 128:(c3 + 1) * 128], pfT_s_a[:],
                         start=False, stop=False)
                T.matmul(pq[:, c3 * 128:(c3 + 1) * 128],
                         pdb[:, c3 * 128:(c3 + 1) * 128], pfT_s_b[:],
                         start=False, stop=True)
        else:
            for c3 in range(3):
                T.matmul(pq[:, c3 * 128:(c3 + 1) * 128],
                         pda[:, c3 * 128:(c3 + 1) * 128], pfT_s_a[:],
                         start=True, stop=False)
                T.matmul(pq[:, c3 * 128:(c3 + 1) * 128],
                         pdb[:, c3 * 128:(c3 + 1) * 128], pfT_s_b[:],
                         start=False, stop=True)
        S.copy(vp16[i][:], pq[:, 0:384])

    def stage_a_flame(h):
        sdt = sd_f_t[:, h * 1152:(h + 1) * 1152]
        pdf = pd_f_t[:, h * 384:(h + 1) * 384]
        pq = psum.tile([128, 512], F32, tag="ps", bufs=2)
        for c3 in range(3):
            for lk in range(3):
                T.matmul(pq[:, c3 * 128:(c3 + 1) * 128],
                         sdt[:, (c3 * 3 + lk) * 128:(c3 * 3 + lk + 1) * 128],
                         betaT_f[:, lk * 128:(lk + 1) * 128],
                         start=(lk == 0), stop=False)
            T.matmul(pq[:, c3 * 128:(c3 + 1) * 128],
                     pdf[:, c3 * 128:(c3 + 1) * 128], pfT_f[:],
                     start=False, stop=True)
        S.copy(vpf16[h][:], pq[:, 0:384])

    def stage_a_mano(h):
        sdt = sd_m_t[:, h * 384:(h + 1) * 384]
        pq = psum.tile([128, 512], F32, tag="ps", bufs=2)
        for c3 in range(3):
            T.matmul(pq[:, 384 + c3:385 + c3], sdt[:, c3 * 128:(c3 + 1) * 128],
                     betam, start=True, stop=True)
        pda = pd_ma_t[:, h * 384:(h + 1) * 384]
        pdb = pd_mb_t[:, h * 384:(h + 1) * 384]
        for c3 in range(3):
            T.matmul(pq[:, c3 * 128:(c3 + 1) * 128],
                     pda[:, c3 * 128:(c3 + 1) * 128], pfT_m_a[h][:],
                     start=True, stop=False)
            T.matmul(pq[:, c3 * 128:(c3 + 1) * 128],
                     pdb[:, c3 * 128:(c3 + 1) * 128], pfT_m_b[h][:],
                     start=False, stop=True)
        vshm = ptile((128, 3), f"vshm{h}")
        S.copy(vshm[:], pq[:, 384:387])
        for c3 in range(3):
            S.add(vpm16[h][:, c3 * 128:(c3 + 1) * 128],
                  pq[:, c3 * 128:(c3 + 1) * 128], vshm[:, c3:c3 + 1])

    for i in range(NCH_PLAIN - 1):
        stage_a_chunk(i)

    # ---------------- rhs64: 12 transposes [B,64] -> [64,128] -------------
    # (GpSimd cannot read PSUM: copies go to Scalar/Vector only.)
    rhs = ptile((NJ64, 1536), "rhs", F16)
    for n4 in range(4):
        for m3 in range(3):
            pp = psum.tile([128, 512], F32, tag="ps", bufs=2)
            src = A4[:, 0:NJ64, m3, n4]
            T.matmul(pp[0:NJ64, 0:128], src, ident, is_transpose=True,
                     start=True, stop=True)
            dst = rhs[0:NJ64, n4 * 384 + m3 * 128:n4 * 384 + (m3 + 1) * 128]
            if (n4 * 3 + m3) % 3 == 2:
                V.tensor_copy(dst, pp[0:NJ64, 0:128])
            else:
                S.copy(dst, pp[0:NJ64, 0:128])



    # ---------------- skinning ---------------------------------------------
    # apply: out = sum_c tp[c,m]*x[c] + tp[3,m], tree-structured adds.
    def t_apply(eng, dst_ap, tp_ap, x16_ap, tag="pr", dt=F16):
        tp4 = tp_ap.rearrange("p (n m b) -> p n m b", m=3, b=128)
        d3 = dst_ap.rearrange("p (m b) -> p m b", b=128)
        x4 = x16_ap.rearrange("p (c b) -> p c b", b=128).unsqueeze(2)
        pr_t = slabs.tile((128, 1152), dt, tag=tag, bufs=2)
        sc_t = slabs.tile((128, 384), dt, tag=tag + "s", bufs=2)
        pr = pr_t[:].rearrange("p (n m b) -> p n m b", m=3, b=128)
        s3 = sc_t[:].rearrange("p (m b) -> p m b", b=128)
        eng.tensor_mul(pr, tp4[:, 0:3], x4.broadcast_to([128, 3, 3, 128]))
        eng.tensor_add(s3, pr[:, 0], pr[:, 1])
        eng.tensor_add(d3, pr[:, 2], tp4[:, 3])
        eng.tensor_add(d3, d3, s3)

    def skin_mm(wt, nrows):
        tpm = psum.tile([128, 1536], F32, tag="skin", bufs=2)
        for g in range(3):
            T.matmul(tpm[:, g * 512:(g + 1) * 512], wt,
                     rhs[0:nrows, g * 512:(g + 1) * 512], start=True, stop=True)
        return tpm

    def skin_hv(i):
        # flame/mano skinning: fp32 copy + GpSimd apply (slow but off the
        # Vector critical path; issued first so latency hides)
        if CH_HEAD0 <= i < CH_HEAD0 + 3:
            h = i - CH_HEAD0
            tpm = skin_mm(w_f_t[:, h * 128:(h + 1) * 128], NJ64)
            xv = vpf16[h]
        else:
            h = i - CH_HL
            tpm = skin_mm(w_m_t[:, h * 128:(h + 1) * 128], NJ64)
            xv = vpm16[h]
        tp32 = slabs.tile((128, 1536), F32, tag="tpx", bufs=2)
        S.copy(tp32[:], tpm[:])
        hv = slabs.tile((128, 384), F16, tag="hv", bufs=2)
        t_apply(P, hv[:], tp32[:], xv[:], tag="prp")
        P.tensor_add(vp16[i][:], vp16[i][:], hv[:])

    def skin_sx(i):
        tpm = skin_mm(w_s_t[:, i * 128:(i + 1) * 128], 22)
        ot = slabs.tile((128, 384), F16, tag="outt", bufs=3)
        tp16 = slabs.tile((128, 1536), F16, tag="tps", bufs=4)
        S.copy(tp16[:], tpm[:])
        t_apply(V, ot[:], tp16[:], vp16[i][:])
        DMA.dma_start(out_d[i * 128:(i + 1) * 128, :], ot[:])

    # fill the FK-wait window with the remaining stage-A PE work, then
    # issue the hv sets first so GpSimd runs them in parallel with the
    # Vector plain applies.
    for i in range(NCH_PLAIN, NCH):
        stage_a_chunk(i)
    for h in range(3):
        stage_a_flame(h)
    for h in range(2):
        transpose_to(pfT_m_a[h][:], pf_m[h][:, 0:128])
        transpose_to(pfT_m_b[h][:], pf_m[h][:, 128:PD_M_K])
    for h in range(2):
        stage_a_mano(h)
    stage_a_chunk(NCH_PLAIN - 1)
    for i in range(CH_HEAD0, NCH):
        skin_hv(i)
    for i in range(NCH_PLAIN):
        skin_sx(i)
    for i in range(CH_HEAD0, NCH):
        skin_sx(i)

    if dbg_d is not None:
        dbg = ptile((128, 2560), "dbg")
        G.memset(dbg[:], 0.0)
        V.tensor_copy(dbg[:, 0:768], Ab[:])
        S.copy(dbg[0:NJ64, 768:768 + 1536], rhs[:])
        S.copy(dbg[:, 2304:2304 + 165], jb[:])
        S.copy(dbg[:, 2469:2469 + 15], jfb[:])
        S.copy(dbg[:, 2484:2484 + 9], bias9[:])
        DMA.dma_start(dbg_d[:], dbg[:])

    es.close()


def _rodrigues(nc, aa, rot, ptile, zero_t):
    """Rodrigues for 55 rots: V handles joints 0:25, GpSimd 25:55; the
    sqrt/sin/cos/reciprocal activations run once on Scalar for all joints."""
    V, S, P = nc.vector, nc.scalar, nc.gpsimd
    J = 55
    aa3 = aa.rearrange("p (j k) -> p j k", k=3)
    sq = ptile((B, J), "rg_sq")
    tmp = ptile((B, 2 * J), "rg_tmp")
    eps_t = ptile((B, 1), "rg_eps")
    P.memset(eps_t[:], 1e-8)
    hpi_t = ptile((B, 1), "rg_hpi")
    P.memset(hpi_t[:], float(np.pi / 2))
    ang = ptile((B, J), "rg_ang")
    inv = ptile((B, J), "rg_inv")
    sn = ptile((B, J), "rg_sin")
    co = ptile((B, J), "rg_cos")
    nv = ptile((B, 3 * J), "rg_n")
    u = ptile((B, J), "rg_u")
    un = ptile((B, 3 * J), "rg_un")
    q = ptile((B, 3 * J), "rg_q")
    d = ptile((B, J), "rg_d")
    dd = ptile((B, J), "rg_dd")
    snv = ptile((B, 3 * J), "rg_snv")
    pp_ = ptile((B, 2 * J), "rg_p")
    r4 = rot[:].rearrange("p (j m n) -> p j m n", m=3, n=3)

    t0 = tmp[:, 0:J]
    V.tensor_mul(sq[:], aa3[:, :, 0], aa3[:, :, 0])
    V.tensor_mul(t0, aa3[:, :, 1], aa3[:, :, 1])
    V.tensor_add(sq[:], sq[:], t0)
    V.tensor_mul(t0, aa3[:, :, 2], aa3[:, :, 2])
    V.tensor_add(sq[:], sq[:], t0)

    S.activation(ang[:], sq[:], AF.Sqrt, bias=eps_t[:])
    S.activation(sn[:], ang[:], AF.Sin, bias=zero_t[:])
    S.activation(co[:], ang[:], AF.Sin, bias=hpi_t[:])
    V.reciprocal(inv[:], ang[:])

    for eng, a, b in ((V, 0, 25), (P, 25, J)):
        n_ = b - a
        n3 = nv[:].rearrange("p (j k) -> p j k", k=3)[:, a:b]
        un3 = un[:].rearrange("p (j k) -> p j k", k=3)[:, a:b]
        q3 = q[:].rearrange("p (j k) -> p j k", k=3)[:, a:b]
        s3 = snv[:].rearrange("p (j k) -> p j k", k=3)[:, a:b]
        eng.tensor_mul(n3, aa3[:, a:b],
                       inv[:, a:b].unsqueeze(2).broadcast_to([B, n_, 3]))
        eng.tensor_scalar(u[:, a:b], co[:, a:b], -1.0, 1.0, ALU.mult, ALU.add)
        eng.tensor_mul(un3, n3, u[:, a:b].unsqueeze(2).broadcast_to([B, n_, 3]))
        eng.tensor_mul(q3, un3, n3)
        eng.tensor_add(d[:, a:b], q3[:, :, 0], q3[:, :, 1])
        eng.tensor_add(d[:, a:b], d[:, a:b], q3[:, :, 2])
        eng.tensor_scalar(dd[:, a:b], d[:, a:b], -1.0, 1.0, ALU.mult, ALU.add)
        eng.tensor_mul(s3, n3, sn[:, a:b].unsqueeze(2).broadcast_to([B, n_, 3]))
        for m in range(3):
            eng.tensor_add(r4[:, a:b, m, m], q3[:, :, m], dd[:, a:b])
        p_ = pp_[:, a:b]
        eng.tensor_mul(p_, un3[:, :, 0], n3[:, :, 1])
        eng.tensor_sub(r4[:, a:b, 0, 1], p_, s3[:, :, 2])
        eng.tensor_add(r4[:, a:b, 1, 0], p_, s3[:, :, 2])
        eng.tensor_mul(p_, un3[:, :, 0], n3[:, :, 2])
        eng.tensor_add(r4[:, a:b, 0, 2], p_, s3[:, :, 1])
        eng.tensor_sub(r4[:, a:b, 2, 0], p_, s3[:, :, 1])
        eng.tensor_mul(p_, un3[:, :, 1], n3[:, :, 2])
        eng.tensor_sub(r4[:, a:b, 1, 2], p_, s3[:, :, 0])
        eng.tensor_add(r4[:, a:b, 2, 1], p_, s3[:, :, 0])


# ================================================================ entry

_CACHED = {}
PROFILE = False
DEBUG = False


def _get_nc():
    if "nc" not in _CACHED:
        _CACHED["nc"] = _build_nc()
    return _CACHED["nc"]


def kernel(**inputs):
    in_maps, vid_all = _host_prep(inputs)
    nc = _get_nc()
    res = run_bass_kernel_spmd(nc, in_maps, core_ids=list(range(NCORES)),
                               trace=PROFILE)
    _CACHED["last_res"] = res
    out = np.zeros((B, VS, 3), np.float32)
    for c in range(NCORES):
        o = np.asarray(res.results[c]["out"]).astype(np.float32).reshape(ROWS, 3, B)
        vok = vid_all[c] >= 0
        out[:, vid_all[c][vok], :] = o[vok].transpose(2, 0, 1)
    return out


# revision 29
# speedup vs baseline: 1.1188x; 1.1188x over previous
"""EHM (SMPLX body + FLAME head + MANO hands) Bass kernel for 8 TRN2 NeuronCores.

Sharding: VERTEX sharding - each core owns 1/8 of the SMPLX vertices (plus the
FLAME/MANO vertices its SMPLX rows stitch in) and computes ALL B=128 batch
elements for its shard.

v3: identity-subtree folding + M-form FK.
  * SMPLX joints 22-54 have zero pose, so A_rel(j) == A_rel(posed ancestor).
    LBS weights fold on host: 55 joints -> 22.  FLAME neck/root and MANO root
    rows are the identity affine; jaw/eyes and hand-d1 joints need no chain
    composition at all.
  * A_rel(j) = A_rel(par) o M_j with M_j = [R_rel | (I-R_rel) J_j]: the rel-
    translation correction ("corr" pass) folds into the per-joint matrix.
  * One unified 64-row rhs [joint, (n4,m3,b)] feeds all skinning matmuls;
    scale/mirror/bias folds and the eyelid offsets live in virtual rows 55-63.
  * Arm chains split via pair-products (X = M_a o M_b) so Vector and GpSimd
    compose in parallel; hand levels are single strided runs.
  * Skinning tail: PSUM->fp16 copies split Scalar/GpSimd, applies split
    Vector/GpSimd.  Inputs stream per-chunk through separate DMA tiles.

Per-vertex data layout: [vertex(partition<=128), (c, b)] with c-major free dim
(col = c*128 + b).  Batch-staged data (poses, FK, A matrices): [b(part), free].
"""

import sys

sys.path.insert(0, "/opt/trn_rl_repo")

from contextlib import ExitStack

import numpy as np
import ml_dtypes

BF16NP = ml_dtypes.bfloat16
F16NP = np.float16

import concourse.bass as bass
import concourse.bacc as bacc
import concourse.tile as tile
import concourse.mybir as mybir
from concourse.bass_utils import run_bass_kernel_spmd

F32 = mybir.dt.float32
BF16 = mybir.dt.bfloat16
F16 = mybir.dt.float16
AF = mybir.ActivationFunctionType
ALU = mybir.AluOpType

# ---------------------------------------------------------------- constants
B = 128
VS, VF, VM = 10475, 5023, 778
NL = 350
NCORES = 8

SMPLX_PARENTS = np.array([-1,0,0,0,1,2,3,4,5,6,7,8,9,9,9,12,13,14,16,17,18,19,
                          15,15,15,20,25,26,20,28,29,20,31,32,20,34,35,20,37,38,
                          21,40,41,21,43,44,21,46,47,21,49,50,21,52,53])
MANO_PARENTS = np.array([-1,0,1,2,0,4,5,0,7,8,0,10,11,0,13,14])
HAND_LVL = [1,4,7,10,13, 2,5,8,11,14, 3,6,9,12,15]   # mano joints, level-major

N_PLAIN, N_HEAD, N_HL, N_HR = 768, 384, 128, 128
ROWS = N_PLAIN + N_HEAD + N_HL + N_HR        # 1408
NCH = ROWS // 128                            # 11
NCH_PLAIN = 6
CH_PLAIN = set(range(0, NCH_PLAIN))
CH_HEAD0 = 6                                 # chunks 6,7,8 head; 9 L; 10 R
CH_HL, CH_HR = 9, 10

PD_S_K = 189
PD_F_K = 27
PD_M_K = 135

# unified A/rhs row map (64 rows):
# 0:22 body | 22:25 flame jaw,eyeL,eyeR | 25:30 Ld1 | 30:35 Rd1 | 35:40 Ld2
# | 40:45 Rd2 | 45:50 Ld3 | 50:55 Rd3 | 55 flameId | 56 Lid | 57 Rid
# | 58:61 r_eyelid rows | 61:64 l_eyelid rows
NJ64 = 64

BF16_INPUTS = {"sd_s0", "sd_s1", "sd_s2", "pd_s_a", "pd_s_b", "sd_f", "pd_f",
               "sd_m", "pd_m_a", "pd_m_b", "cb_b"}
F16_INPUTS = {"w_s", "w_f", "w_m"}


# ================================================================ host prep

def _split_sizes(total, parts):
    q, r = divmod(total, parts)
    return [q + (1 if i < r else 0) for i in range(parts)]


def _pad_ids(ids, n):
    out = np.full(n, -1, np.int64)
    out[:len(ids)] = ids
    return out


def _host_prep(inp):
    f32 = np.float32
    s2f = np.asarray(inp["smplx2flame_ind"])
    head_ix = np.asarray(inp["head_index"])
    s2l = np.asarray(inp["smplx2mano_left"])
    s2r = np.asarray(inp["smplx2mano_right"])

    head_sv = s2f[head_ix]
    special = np.zeros(VS, bool)
    special[head_sv] = True
    special[s2l] = True
    special[s2r] = True
    plain_sv = np.nonzero(~special)[0]

    pl_sp = np.cumsum([0] + _split_sizes(len(plain_sv), NCORES))
    hd_sp = np.cumsum([0] + _split_sizes(len(head_ix), NCORES))
    hl_sp = np.cumsum([0] + _split_sizes(VM, NCORES))

    sd_s_np = np.asarray(inp["smplx_shapedirs"], f32)
    pd_s_np = np.asarray(inp["smplx_posedirs"], f32)
    jr_s_np = np.asarray(inp["smplx_J_regressor"], f32)
    w_s_np = np.asarray(inp["smplx_lbs_weights"], f32)
    tmpl_s = np.asarray(inp["smplx_v_template"], f32)
    sd_f_np = np.asarray(inp["flame_shapedirs"], f32)
    pd_f_np = np.asarray(inp["flame_posedirs"], f32)
    jr_f_np = np.asarray(inp["flame_J_regressor"], f32)
    w_f_np = np.asarray(inp["flame_lbs_weights"], f32)
    tmpl_f = np.asarray(inp["flame_v_template"], f32)
    re_np = np.asarray(inp["r_eyelid"], f32)
    le_np = np.asarray(inp["l_eyelid"], f32)
    sd_m_np = np.asarray(inp["mano_shapedirs"], f32)
    pd_m_np = np.asarray(inp["mano_posedirs"], f32)
    jr_m_np = np.asarray(inp["mano_J_regressor"], f32)
    w_m_np = np.asarray(inp["mano_lbs_weights"], f32)
    tmpl_m = np.asarray(inp["mano_v_template"], f32)

    # folded smplx weights: 55 -> 22
    w_fold = w_s_np[:, 0:22].copy()
    w_fold[:, 15] += w_s_np[:, 22:25].sum(1)
    w_fold[:, 20] += w_s_np[:, 25:40].sum(1)
    w_fold[:, 21] += w_s_np[:, 40:55].sum(1)

    # pose vector in v3 55-joint order
    lh = np.asarray(inp["left_hand_pose"], f32).reshape(B, 15, 3)
    rh = np.asarray(inp["right_hand_pose"], f32).reshape(B, 15, 3)
    lvl = [j - 1 for j in HAND_LVL]
    aa = np.concatenate([
        np.asarray(inp["global_pose"], f32).reshape(B, 3),
        np.asarray(inp["body_pose"], f32).reshape(B, 63),
        np.asarray(inp["jaw_params"], f32).reshape(B, 3),
        np.asarray(inp["eye_pose"], f32).reshape(B, 6),
        lh[:, lvl[0:5]].reshape(B, 15), rh[:, lvl[0:5]].reshape(B, 15),
        lh[:, lvl[5:10]].reshape(B, 15), rh[:, lvl[5:10]].reshape(B, 15),
        lh[:, lvl[10:15]].reshape(B, 15), rh[:, lvl[10:15]].reshape(B, 15),
    ], axis=1)                                               # [B,165]

    ep = np.asarray(inp["eyelid_params"], f32)
    aux = np.concatenate([
        np.asarray(inp["head_scale"], f32)[:, None],
        np.asarray(inp["left_hand_scale"], f32)[:, None],
        np.asarray(inp["right_hand_scale"], f32)[:, None],
        ep[:, 0:1], ep[:, 1:2],
        np.asarray(inp["head_pos_offset"], f32),
        np.asarray(inp["left_hand_pos_offset"], f32),
        np.asarray(inp["right_hand_pos_offset"], f32),
    ], axis=1)                                               # [128, 14]

    def beta_T(second):
        b = np.concatenate([np.asarray(inp["shape_params"], f32), second], 1)
        bt = np.zeros((384, B), f32)
        bt[:NL] = b.T
        bt[NL] = 1.0
        return bt.reshape(3, 128, B)

    betaT_s = beta_T(np.asarray(inp["body_exp"], f32))
    betaT_f = beta_T(np.asarray(inp["flame_exp"], f32))

    joff = np.asarray(inp["joints_offset"], f32)

    # ---- J regression folded into weights: Jmat = Jreg @ [shapedirs | tmpl] ----
    def jdirs(jr, sd, tmpl, nj):
        ext = np.concatenate([sd, tmpl[:, :, None]], axis=2)      # [V,3,L+1]
        jm = jr @ ext.reshape(ext.shape[0], -1)                   # [nj, 3*(L+1)]
        return jm.reshape(nj, 3, ext.shape[2])

    jm_s = jdirs(jr_s_np, sd_s_np, tmpl_s, 55)                    # [55,3,351]
    jm_f = jdirs(jr_f_np, sd_f_np, tmpl_f, 5)                     # [5,3,351]
    jm_m = jdirs(jr_m_np, sd_m_np, tmpl_m, 16)                    # [16,3,11]

    # device lhsT layout: jd[lk, l, c*nj + j] = jm[j, c, lk*128 + l]
    def jd_pack(jm, nj):
        out = np.zeros((3, 128, 3 * nj), f32)
        L = jm.shape[2]
        for lk in range(3):
            l0, l1 = lk * 128, min((lk + 1) * 128, L)
            blk = jm[:, :, l0:l1]                                 # [nj,3,n]
            out[lk, :l1 - l0] = blk.transpose(2, 1, 0).reshape(l1 - l0, 3 * nj)
        return out

    jd_s = jd_pack(jm_s, 55)
    jd_f = jd_pack(jm_f, 5)

    # MANO joints are batch-independent (shared betas): compute on host.
    bm_ext = np.concatenate([np.asarray(inp["mano_betas"], f32)[0], [1.0]])
    jmano = np.einsum('jcl,l->jc', jm_m, bm_ext)                  # [16,3]
    # level-major J for hands (same for L and R), c-major: [3,15] -> 45
    jm_lvl = jmano[HAND_LVL]                                      # [15,3]
    jmb = np.broadcast_to(jm_lvl.T.reshape(1, 45), (B, 45)).copy()
    jm0 = np.broadcast_to(jmano[0][None], (B, 3)).copy()

    betam = np.zeros((11, 1), f32)
    betam[:10, 0] = np.asarray(inp["mano_betas"], f32)[0]
    betam[10, 0] = 1.0

    # joints_offset in batch layout
    joffb = np.ascontiguousarray(joff.transpose(0, 2, 1)).reshape(B, 165)

    # pack all small constants: cb_p (pose; tiny, lands first) + cb_c
    cb_p = np.zeros((128, 179), f32)
    cb_p[:, 0:165] = aa
    cb_p[:, 165:179] = aux
    cb_c = np.zeros((128, 341), f32)
    cb_c[:, 0:128] = np.eye(128, dtype=f32)
    cb_c[:, 128:293] = joffb
    cb_c[:, 293:338] = jmb
    cb_c[:, 338:341] = jm0
    cb_b = np.zeros((128, 1309), f32)
    cb_b[:, 0:384] = betaT_s.transpose(1, 0, 2).reshape(128, 384)
    cb_b[:, 384:768] = betaT_f.transpose(1, 0, 2).reshape(128, 384)
    cb_b[:, 768:1263] = jd_s.transpose(1, 0, 2).reshape(128, 495)
    cb_b[:, 1263:1308] = jd_f.transpose(1, 0, 2).reshape(128, 45)
    cb_b[0:11, 1308] = betam[:, 0]
    rep = dict(cb_p=cb_p, cb_c=cb_c, cb_b=cb_b)

    # mano posedirs rows permuted to level-major order
    pd_m_perm = np.empty_like(pd_m_np)
    for pos, j in enumerate(HAND_LVL):
        pd_m_perm[pos * 9:(pos + 1) * 9] = pd_m_np[(j - 1) * 9:j * 9]

    in_maps = []
    vid_all = np.full((NCORES, ROWS), -1, np.int64)

    for c in range(NCORES):
        p_ids = plain_sv[pl_sp[c]:pl_sp[c + 1]]
        h_pos = np.arange(hd_sp[c], hd_sp[c + 1])
        h_sv, h_fv = head_sv[h_pos], head_ix[h_pos]
        l_pos = np.arange(hl_sp[c], hl_sp[c + 1])
        r_pos = l_pos
        l_sv, r_sv = s2l[l_pos], s2r[r_pos]

        vid = np.full(ROWS, -1, np.int64)
        vid[:len(p_ids)] = p_ids
        vid[N_PLAIN:N_PLAIN + len(h_sv)] = h_sv
        vid[N_PLAIN + N_HEAD:N_PLAIN + N_HEAD + len(l_sv)] = l_sv
        vid[N_PLAIN + N_HEAD + N_HL:N_PLAIN + N_HEAD + N_HL + len(r_sv)] = r_sv
        vid_all[c] = vid
        vok = vid >= 0
        vc = np.where(vok, vid, 0)

        # smplx shapedirs slab, PLAIN chunks only: [6, 128(p=l), (c, lk, v)]
        pvc = vc[:N_PLAIN]
        pvok = vok[:N_PLAIN]
        sdp = np.zeros((N_PLAIN, 3, 384), f32)
        sdp[:, :, :NL] = np.where(pvok[:, None, None], sd_s_np[pvc], 0.0)
        sdp[:, :, NL] = np.where(pvok[:, None], tmpl_s[pvc], 0.0)
        slab = sdp.reshape(NCH_PLAIN, 128, 3, 3, 128).transpose(0, 4, 2, 3, 1)
        sd_s = np.ascontiguousarray(slab).reshape(NCH_PLAIN, 128, 1152)

        # smplx posedirs, all chunks
        colv = vc[:, None] * 3 + np.arange(3)[None, :]
        pdv = pd_s_np[:PD_S_K][:, colv]
        pdv = np.where(vok[None, :, None], pdv, 0.0)
        pdv = pdv.reshape(PD_S_K, NCH, 128, 3).transpose(1, 0, 3, 2)
        pd_s_a = np.ascontiguousarray(pdv[:, :128]).reshape(NCH, 128, 384)
        pd_s_b = np.ascontiguousarray(pdv[:, 128:]).reshape(NCH, PD_S_K - 128, 384)

        # folded smplx weights stationary [22, NCH*128]
        w_s = np.ascontiguousarray(
            np.where(vok[:, None], w_fold[vc], 0.0)
            .reshape(NCH, 128, 22).transpose(0, 2, 1))

        # flame: 3 gathered head chunks only
        fg = _pad_ids(h_fv, N_HEAD)
        fok = fg >= 0
        fc = np.where(fok, fg, 0)
        sdfp = np.zeros((N_HEAD, 3, 384), f32)
        sdfp[:, :, :NL] = np.where(fok[:, None, None], sd_f_np[fc], 0.0)
        sdfp[:, :, NL] = np.where(fok[:, None], tmpl_f[fc], 0.0)
        slab = sdfp.reshape(3, 128, 3, 3, 128).transpose(0, 4, 2, 3, 1)
        sd_f = np.ascontiguousarray(slab).reshape(3, 128, 1152)

        colf = fc[:, None] * 3 + np.arange(3)[None, :]
        pdfv = pd_f_np[9:36][:, colf]
        pdfv = np.where(fok[None, :, None], pdfv, 0.0)
        pdfv = pdfv.reshape(PD_F_K, 3, 128, 3).transpose(1, 0, 3, 2)
        pd_f = np.ascontiguousarray(pdfv).reshape(3, PD_F_K, 384)

        # flame stationary [64, 3*128]: jaw/eyes, id row, eyelid rows
        w_f = np.zeros((3, NJ64, 128), f32)
        for k in range(3):
            rows, ok = fc[k * 128:(k + 1) * 128], fok[k * 128:(k + 1) * 128]
            w_f[k, 22:25] = np.where(ok[None, :], w_f_np[rows][:, 2:5].T, 0.0)
            w_f[k, 55] = np.where(ok, w_f_np[rows][:, 0] + w_f_np[rows][:, 1], 0.0)
            w_f[k, 58:61] = np.where(ok[None, :], re_np[rows].T, 0.0)
            w_f[k, 61:64] = np.where(ok[None, :], le_np[rows].T, 0.0)

        # mano hands
        m_rows = np.stack([_pad_ids(l_pos, 128), _pad_ids(r_pos, 128)])
        mok = m_rows >= 0
        mc = np.where(mok, m_rows, 0)
        sd_m = np.zeros((2, 11, 384), f32)
        pd_m_a = np.zeros((2, 128, 384), f32)
        pd_m_b = np.zeros((2, PD_M_K - 128, 384), f32)
        w_m = np.zeros((2, NJ64, 128), f32)
        for h in range(2):
            sdm = np.where(mok[h][:, None, None], sd_m_np[mc[h]], 0.0)
            sd_m[h, :10] = sdm.transpose(2, 1, 0).reshape(10, 384)
            sd_m[h, 10] = np.where(mok[h][:, None], tmpl_m[mc[h]], 0.0).T.reshape(384)
            colm = mc[h][:, None] * 3 + np.arange(3)[None, :]
            pdm = pd_m_perm[:, colm]
            pdm = np.where(mok[h][None, :, None], pdm, 0.0).transpose(0, 2, 1)
            pd_m_a[h] = pdm[:128].reshape(128, 384)
            pd_m_b[h] = pdm[128:].reshape(PD_M_K - 128, 384)
            wm = np.where(mok[h][None, :], w_m_np[mc[h]].T, 0.0)  # [16,128]
            # posed rows: level-major, interleaved L/R blocks
            for lv in range(3):
                r0 = 25 + lv * 10 + h * 5
                w_m[h, r0:r0 + 5] = wm[HAND_LVL[lv * 5:(lv + 1) * 5]]
            w_m[h, 56 + h] = wm[0]

        m = dict(rep)
        pk = lambda a: np.ascontiguousarray(a.transpose(1, 0, 2)).reshape(a.shape[1], -1)
        m.update(sd_s0=sd_s[0:2].transpose(1, 0, 2).reshape(128, -1),
                 sd_s1=sd_s[2:4].transpose(1, 0, 2).reshape(128, -1),
                 sd_s2=sd_s[4:6].transpose(1, 0, 2).reshape(128, -1),
                 pd_s_a=pk(pd_s_a), pd_s_b=pk(pd_s_b), w_s=pk(w_s),
                 sd_f=pk(sd_f), pd_f=pk(pd_f), w_f=pk(w_f),
                 sd_m=pk(sd_m), pd_m_a=pk(pd_m_a), pd_m_b=pk(pd_m_b), w_m=pk(w_m))
        out = {}
        for k, v in m.items():
            v = np.ascontiguousarray(v)
            if k in BF16_INPUTS:
                out[k] = np.ascontiguousarray(v.astype(BF16NP))
            elif k in F16_INPUTS:
                out[k] = np.ascontiguousarray(v.astype(F16NP))
            else:
                out[k] = np.ascontiguousarray(v, f32)
        in_maps.append(out)

    return in_maps, vid_all


# ================================================================ device IR

def _build_nc():
    nc = bacc.Bacc("TRN2", target_bir_lowering=False, debug=False,
                   num_devices=NCORES)
    di = {}

    def din(name, shape):
        dt = BF16 if name in BF16_INPUTS else (F16 if name in F16_INPUTS else F32)
        di[name] = nc.dram_tensor(name, list(shape), dt, kind="ExternalInput").ap()

    din("cb_p", (128, 179)); din("cb_c", (128, 341))
    din("cb_b", (128, 1309))
    din("sd_s0", (128, 2304)); din("sd_s1", (128, 2304)); din("sd_s2", (128, 2304))
    din("pd_s_a", (128, NCH * 384)); din("pd_s_b", (PD_S_K - 128, NCH * 384))
    din("w_s", (22, NCH * 128))
    din("sd_f", (128, 3 * 1152)); din("pd_f", (PD_F_K, 3 * 384))
    din("w_f", (NJ64, 3 * 128))
    din("sd_m", (11, 2 * 384)); din("pd_m_a", (128, 2 * 384))
    din("pd_m_b", (PD_M_K - 128, 2 * 384)); din("w_m", (NJ64, 2 * 128))

    out_d = nc.dram_tensor("out", [ROWS, 384], F16, kind="ExternalOutput").ap()
    dbg_d = None
    if DEBUG:
        dbg_d = nc.dram_tensor("dbg", [128, 2560], F32, kind="ExternalOutput").ap()

    with tile.TileContext(nc) as tc:
        _emit(nc, tc, di, out_d, dbg_d)
    nc.compile()
    return nc


def _emit(nc, tc, di, out_d, dbg_d=None):
    es = ExitStack()
    persist = es.enter_context(tc.tile_pool(name="persist", bufs=1))
    slabs = es.enter_context(tc.tile_pool(name="slabs", bufs=3))
    psum = es.enter_context(tc.tile_pool(name="psum", bufs=1, space="PSUM"))

    V, S, G, T, DMA = nc.vector, nc.scalar, nc.gpsimd, nc.tensor, nc.sync
    P = nc.gpsimd

    def ptile(shape, name, dt=F32):
        return persist.tile(list(shape), dt, tag=name, name=name)

    # ---------------- DMAs: constants first, then streaming slabs ---------
    cb_p = ptile((128, 179), "cb_p")
    DMA.dma_start(cb_p[:], di["cb_p"][:])
    cb_b = ptile((128, 1309), "cb_b", BF16)
    DMA.dma_start(cb_b[:], di["cb_b"][:])
    cb_c = ptile((128, 341), "cb_c")
    DMA.dma_start(cb_c[:], di["cb_c"][:])

    aa = cb_p[:][:, 0:165]
    aux = cb_p[:][:, 165:179]
    c32 = cb_c[:]
    ident = c32[:, 0:128]
    joffb = c32[:, 128:293]
    jmb = c32[:, 293:338]       # [B, (c,15)] level-major hand J (L==R)
    jm0 = c32[:, 338:341]
    c16 = cb_b[:]
    betaT_s = c16[:, 0:384]
    betaT_f = c16[:, 384:768]
    jd_s = c16[:, 768:1263]
    jd_f = c16[:, 1263:1308]
    betam = cb_b[0:11, 1308:1309]

    # activation-table preload: two dummy activations on Scalar before its
    # DMA issues, so sqrt/sin tables are resident before rodrigues needs them
    dum = ptile((128, 4), "dum")
    zero_t = ptile((B, 1), "rg_zero")
    G.memset(zero_t[:], 0.0)
    G.memset(dum[:], 0.0)
    S.activation(dum[:, 0:2], dum[:, 2:4], AF.Sin, bias=zero_t[:])
    S.activation(dum[:, 0:2], dum[:, 2:4], AF.Sqrt, bias=zero_t[:])

    # stage-A-critical inputs on the sync queue, in consumption order
    sd_s_t = [ptile((128, 2304), f"sd_s{i}", BF16) for i in range(3)]
    pd_a_t = ptile((128, NCH * 384), "pd_a", BF16)
    pd_b_t = ptile((PD_S_K - 128, NCH * 384), "pd_b", BF16)
    DMA.dma_start(sd_s_t[0][:], di["sd_s0"][:])
    DMA.dma_start(pd_a_t[:, 0:3 * 384], di["pd_s_a"][:, 0:3 * 384])
    DMA.dma_start(pd_b_t[:, 0:3 * 384], di["pd_s_b"][:, 0:3 * 384])
    DMA.dma_start(sd_s_t[1][:], di["sd_s1"][:])
    DMA.dma_start(pd_a_t[:, 3 * 384:6 * 384], di["pd_s_a"][:, 3 * 384:6 * 384])
    DMA.dma_start(pd_b_t[:, 3 * 384:6 * 384], di["pd_s_b"][:, 3 * 384:6 * 384])
    DMA.dma_start(sd_s_t[2][:], di["sd_s2"][:])
    sd_f_t = ptile((128, 3 * 1152), "sd_f", BF16)
    DMA.dma_start(sd_f_t[:], di["sd_f"][:])
    DMA.dma_start(pd_a_t[:, 6 * 384:], di["pd_s_a"][:, 6 * 384:])
    DMA.dma_start(pd_b_t[:, 6 * 384:], di["pd_s_b"][:, 6 * 384:])
    pd_f_t = ptile((PD_F_K, 3 * 384), "pd_f", BF16)
    DMA.dma_start(pd_f_t[:], di["pd_f"][:])

    w_s_t = ptile((22, NCH * 128), "w_s", F16)
    DMA.dma_start(w_s_t[:], di["w_s"][:])
    sd_m_t = ptile((11, 768), "sd_m", BF16)
    DMA.dma_start(sd_m_t[:], di["sd_m"][:])
    pd_ma_t = ptile((128, 768), "pd_ma", BF16)
    DMA.dma_start(pd_ma_t[:], di["pd_m_a"][:])
    pd_mb_t = ptile((PD_M_K - 128, 768), "pd_mb", BF16)
    DMA.dma_start(pd_mb_t[:], di["pd_m_b"][:])
    w_f_t = ptile((NJ64, 384), "w_f", F16)
    DMA.dma_start(w_f_t[:], di["w_f"][:])
    w_m_t = ptile((NJ64, 256), "w_m", F16)
    DMA.dma_start(w_m_t[:], di["w_m"][:])

    # ---------------- joints (6 small MMs, one accumulation group/bank) ---
    jp = psum.tile([128, 512], F32, tag="ps", bufs=2)
    for lk in range(3):
        T.matmul(jp[:, 0:165], betaT_s[:, lk * 128:(lk + 1) * 128],
                 jd_s[:, lk * 165:(lk + 1) * 165],
                 start=(lk == 0), stop=(lk == 2))
    jpf = psum.tile([128, 512], F32, tag="ps", bufs=2)
    for lk in range(3):
        T.matmul(jpf[:, 0:15], betaT_f[:, lk * 128:(lk + 1) * 128],
                 jd_f[:, lk * 15:(lk + 1) * 15],
                 start=(lk == 0), stop=(lk == 2))

    jb = ptile((B, 165), "jb")
    jfb = ptile((B, 15), "jfb")
    V.tensor_add(jb[:], jp[:, 0:165], joffb)
    S.copy(jfb[:], jpf[:, 0:15])

    # ---------------- rodrigues (55 rots, v3 order) -----------------------
    rot = ptile((B, 55 * 9), "rot")
    _rodrigues(nc, aa, rot, ptile, zero_t)
    rot4 = rot[:].rearrange("p (j x) -> p j x", x=9)
    rot5 = rot[:].rearrange("p (j m n) -> p j m n", m=3, n=3)

    # ---------------- pose features + transposes --------------------------
    def pf_diag_sub(t9, n):
        V.tensor_scalar_add(t9[:, :, 0:9:4], t9[:, :, 0:9:4], -1.0)

    pf_s = ptile((B, 189), "pf_s")
    t9 = pf_s[:].rearrange("p (j x) -> p j x", x=9)
    V.tensor_copy(t9, rot4[:, 1:22, :])
    pf_diag_sub(t9, 21)
    pf_f = ptile((B, 27), "pf_f")
    t9 = pf_f[:].rearrange("p (j x) -> p j x", x=9)
    V.tensor_copy(t9, rot4[:, 22:25, :])
    pf_diag_sub(t9, 3)
    # hands: L blocks at lvl*10+0, R at lvl*10+5 (within rows 25:55)
    pf_m = [ptile((B, 135), f"pf_m{h}") for h in range(2)]
    handrot = rot[:, 25 * 9:55 * 9].rearrange("p (l h x) -> p l h x", l=3, h=2)
    for h in range(2):
        t9 = pf_m[h][:].rearrange("p (l x) -> p l x", l=3)
        P.tensor_copy(t9, handrot[:, :, h, :])
        P.tensor_scalar_add(
            pf_m[h][:].rearrange("p (j x) -> p j x", x=9)[:, :, 0:9:4],
            pf_m[h][:].rearrange("p (j x) -> p j x", x=9)[:, :, 0:9:4], -1.0)

    def transpose_to(dst_ap, src_ap):
        pp = psum.tile([128, 512], F32, tag="ps", bufs=2)
        k, n = src_ap.shape[0], src_ap.shape[1]
        T.matmul(pp[:n, :k], src_ap, ident[0:k, 0:k], is_transpose=True,
                 start=True, stop=True)
        S.copy(dst_ap, pp[:n, :k])

    pfT_s_a = ptile((128, 128), "pfT_s_a", BF16)
    pfT_s_b = ptile((PD_S_K - 128, 128), "pfT_s_b", BF16)
    transpose_to(pfT_s_a[:], pf_s[:, 0:128])
    transpose_to(pfT_s_b[:], pf_s[:, 128:PD_S_K])
    pfT_f = ptile((PD_F_K, 128), "pfT_f", BF16)
    transpose_to(pfT_f[:], pf_f[:, :])
    pfT_m_a = [ptile((128, 128), f"pfT_m{h}a", BF16) for h in range(2)]
    pfT_m_b = [ptile((PD_M_K - 128, 128), f"pfT_m{h}b", BF16) for h in range(2)]

    epp = ptile((B, 2), "epp")
    negls = ptile((B, 1), "negls")
    P.tensor_scalar_mul(negls[:], aux[:, 1:2], -1.0)

    # ---------------- M matrices: [R | (I-R) J] ---------------------------
    Mb = ptile((B, 55 * 12), "Mb")
    M4 = Mb[:].rearrange("p (j m n) -> p j m n", m=3, n=4)
    Ab = ptile((B, NJ64 * 12), "Ab")
    A4 = Ab[:].rearrange("p (j m n) -> p j m n", m=3, n=4)
    scr = ptile((B, 264), "scr")    # V-side fk/M scratch
    scrp = ptile((B, 264), "scrp")  # P-side fk/M scratch

    # rotation part (one big copy)
    V.tensor_copy(M4[:, 0:55, :, 0:3], rot5[:, 0:55])

    # translation: t = J - R J.  Jfull is [B, (c, nJ)] c-major; section
    # joint i corresponds to J column c*nJ + joff + i.
    def m_trans(eng, sc_t, j0, nj, Jfull, nJ, joff):
        dst = M4[:, j0:j0 + nj, :, 3]
        sc3 = sc_t[:].rearrange("p (j m) -> p j m", m=3)[:, 0:nj]
        for k in range(3):
            jk = Jfull[:, k * nJ + joff:k * nJ + joff + nj]
            jk = jk.unsqueeze(2).broadcast_to([B, nj, 3])
            rk = rot5[:, j0:j0 + nj, :, k]
            if k == 0:
                eng.tensor_mul(dst, rk, jk)
            else:
                eng.tensor_mul(sc3, rk, jk)
                eng.tensor_add(dst, dst, sc3)
        # dst = J^T(m-major view) - dst
        jm_ = Jfull.rearrange("p (c j) -> p j c", c=3)[:, joff:joff + nj]
        eng.tensor_sub(dst, jm_, dst)

    m_trans(V, scr, 0, 22, jb[:], 55, 0)
    m_trans(V, scr, 22, 3, jfb[:], 5, 2)
    # hands: J = jmb (c-major 15, level-major), shared by L and R.
    # M trans rows 25:55 viewed as [B, lvl(3), hj(10), x(12)]; L = hj 0:5.
    hx = Mb[:, 25 * 12:55 * 12].rearrange("p (l hj x) -> p l hj x", hj=10, x=12)
    hrot = rot[:, 25 * 9:55 * 9].rearrange("p (l hj x) -> p l hj x", hj=10, x=9)
    scp4 = scrp[:, 0:180].rearrange("p (l hj m) -> p l hj m", l=3, m=3)
    jmTv = jmb[:].rearrange("p (m l j) -> p l j m", m=3, l=3)    # [B,3,5,3]
    for h in range(2):
        dsth = hx[:, :, h * 5:(h + 1) * 5, 3:12:4]               # [B,3,5,3]
        sch = scp4[:, :, h * 5:(h + 1) * 5, :]
        for k in range(3):
            jsl = jmb[:, k * 15:(k + 1) * 15].rearrange("p (l j) -> p l j", l=3)
            jsl = jsl.unsqueeze(3).broadcast_to([B, 3, 5, 3])
            rk = hrot[:, :, h * 5:(h + 1) * 5, k:9:3]            # [B,3,5,3] m
            if k == 0:
                P.tensor_mul(dsth, rk, jsl)
            else:
                P.tensor_mul(sch, rk, jsl)
                P.tensor_add(dsth, dsth, sch)
        P.tensor_sub(dsth, jmTv, dsth)

    # ---------------- FK: A = A_par o M -----------------------------------
    G.memset(Ab[:, 55 * 12:NJ64 * 12], 0.0)

    # direct rows: body root, flame jaw/eyes, hands d1 (25:35)
    V.tensor_copy(A4[:, 0:1], M4[:, 0:1])
    V.tensor_copy(A4[:, 22:25], M4[:, 22:25])
    P.tensor_copy(A4[:, 25:35], M4[:, 25:35])

    def compose(eng, dst, par, dT, n, scr_t):
        # dst = par o dT ; par [B,n,3,4] (world), dT [B,n,3,4] (rel)
        sc = scr_t[:].rearrange("p (j m n) -> p j m n", m=3, n=4)[:, :n]
        for k in range(3):
            a_k = par[:, :, :, k:k + 1].broadcast_to([B, n, 3, 4])
            t_k = dT[:, :, k:k + 1, :].broadcast_to([B, n, 3, 4])
            if k == 0:
                eng.tensor_mul(dst, a_k, t_k)
            else:
                eng.tensor_mul(sc, a_k, t_k)
                eng.tensor_add(dst, dst, sc)
        eng.tensor_add(dst[:, :, :, 3], dst[:, :, :, 3], par[:, :, :, 3])

    # pair products on P (independent of trunk): X16,17 = M13,14 o M16,17 ;
    # X20,21 = M18,19 o M20,21
    Xb = ptile((B, 4 * 12), "Xb")
    X4 = Xb[:].rearrange("p (j m n) -> p j m n", m=3, n=4)
    compose(P, X4[:, 0:2], M4[:, 13:15], M4[:, 16:18], 2, scrp)
    compose(P, X4[:, 2:4], M4[:, 18:20], M4[:, 20:22], 2, scrp)
    # hand levels on P: d2 (35:45 <- 25:35), d3 (45:55 <- 35:45)
    compose(P, A4[:, 35:45], A4[:, 25:35], M4[:, 35:45], 10, scrp)
    compose(P, A4[:, 45:55], A4[:, 35:45], M4[:, 45:55], 10, scrp)

    # body trunk on V; arm chain A16,17 = A9 o X1 and A20,21 = A16,17 o X2
    # runs on P so V and P advance in parallel after A9.
    compose(V, A4[:, 1:4], A4[:, 0:1].broadcast_to([B, 3, 3, 4]),
            M4[:, 1:4], 3, scr)
    compose(V, A4[:, 4:7], A4[:, 1:4], M4[:, 4:7], 3, scr)
    compose(V, A4[:, 7:10], A4[:, 4:7], M4[:, 7:10], 3, scr)
    compose(P, A4[:, 16:18], A4[:, 9:10].broadcast_to([B, 2, 3, 4]),
            X4[:, 0:2], 2, scrp)
    compose(P, A4[:, 20:22], A4[:, 16:18], X4[:, 2:4], 2, scrp)
    compose(V, A4[:, 10:13], A4[:, 7:10], M4[:, 10:13], 3, scr)
    compose(V, A4[:, 13:15], A4[:, 9:10].broadcast_to([B, 2, 3, 4]),
            M4[:, 13:15], 2, scr)
    compose(V, A4[:, 15:16], A4[:, 12:13], M4[:, 15:16], 1, scr)
    compose(V, A4[:, 18:20], A4[:, 16:18], M4[:, 18:20], 2, scr)

    # ---------------- biases (need only jb/jfb, run during FK tail) -------
    jb3 = jb[:].rearrange("p (c j) -> p c j", c=3)
    jf3 = jfb[:].rearrange("p (c j) -> p c j", c=3)
    bias9 = ptile((B, 9), "bias9")      # 0:3 flame, 3:6 L, 6:9 R
    hm = ptile((B, 8), "hm")
    V.tensor_add(hm[:, 0:3], jb3[:, :, 23], jb3[:, :, 24])
    V.tensor_add(hm[:, 3:6], jf3[:, :, 3], jf3[:, :, 4])
    V.tensor_sub(hm[:, 0:3], hm[:, 0:3], hm[:, 3:6])
    V.tensor_scalar_mul(hm[:, 0:3], hm[:, 0:3], 0.5)
    V.tensor_add(bias9[:, 0:3], hm[:, 0:3], aux[:, 5:8])
    # bias_l = (lpo - jm0)*[-1,1,1] + tbj20 ; bias_r = rpo - jm0 + tbj21
    V.tensor_sub(hm[:, 3:6], aux[:, 8:11], jm0)
    V.tensor_sub(bias9[:, 3:4], jb3[:, 0:1, 20], hm[:, 3:4])
    V.tensor_add(bias9[:, 4:6], hm[:, 4:6], jb3[:, 1:3, 20])
    V.tensor_sub(hm[:, 3:6], aux[:, 11:14], jm0)
    V.tensor_add(bias9[:, 6:9], hm[:, 3:6], jb3[:, :, 21])
    V.tensor_mul(epp[:], aux[:, 3:5], aux[:, 0:1].broadcast_to([B, 2]))

    # ---------------- folds: scale/mirror/bias + virtual rows -------------
    # flame rows 22:25: A = s*A ; trans += bias_f
    V.tensor_scalar_mul(Ab[:, 22 * 12:25 * 12], Ab[:, 22 * 12:25 * 12],
                        aux[:, 0:1])
    V.tensor_add(A4[:, 22:25, :, 3], A4[:, 22:25, :, 3],
                 bias9[:, 0:3].unsqueeze(1).broadcast_to([B, 3, 3]))
    # hands: rows 25:55 as [B, lvl(3), hj(10), x(12)]; L = hj 0:5, R = 5:10
    hAx = Ab[:, 25 * 12:55 * 12].rearrange("p (l hj x) -> p l hj x",
                                           hj=10, x=12)
    # L: row m=0 (x 0:4) *= -s_l ; m=1,2 (x 4:12) *= s_l ; R: all *= s_r
    P.tensor_mul(hAx[:, :, 0:5, 0:4], hAx[:, :, 0:5, 0:4],
                 negls[:, 0:1].unsqueeze(2).unsqueeze(3)
                 .broadcast_to([B, 3, 5, 4]))
    P.tensor_mul(hAx[:, :, 0:5, 4:12], hAx[:, :, 0:5, 4:12],
                 aux[:, 1:2].unsqueeze(2).unsqueeze(3)
                 .broadcast_to([B, 3, 5, 8]))
    P.tensor_mul(hAx[:, :, 5:10, 0:12], hAx[:, :, 5:10, 0:12],
                 aux[:, 2:3].unsqueeze(2).unsqueeze(3)
                 .broadcast_to([B, 3, 5, 12]))
    # trans += bias  (x = 3,7,11 gives m = 0,1,2)
    P.tensor_add(hAx[:, :, 0:5, 3:12:4], hAx[:, :, 0:5, 3:12:4],
                 bias9[:, 3:6].unsqueeze(1).unsqueeze(2)
                 .broadcast_to([B, 3, 5, 3]))
    P.tensor_add(hAx[:, :, 5:10, 3:12:4], hAx[:, :, 5:10, 3:12:4],
                 bias9[:, 6:9].unsqueeze(1).unsqueeze(2)
                 .broadcast_to([B, 3, 5, 3]))
    # id rows 55,56,57: diag scale + bias trans
    V.tensor_copy(Ab[:, 55 * 12:55 * 12 + 11:5],
                  aux[:, 0:1].broadcast_to([B, 3]))
    V.tensor_copy(A4[:, 55, :, 3], bias9[:, 0:3])
    P.tensor_copy(Ab[:, 56 * 12:56 * 12 + 1], negls[:])
    P.tensor_copy(Ab[:, 56 * 12 + 5:56 * 12 + 11:5],
                  aux[:, 1:2].broadcast_to([B, 2]))
    P.tensor_copy(A4[:, 56, :, 3], bias9[:, 3:6])
    P.tensor_copy(Ab[:, 57 * 12:57 * 12 + 11:5],
                  aux[:, 2:3].broadcast_to([B, 3]))
    P.tensor_copy(A4[:, 57, :, 3], bias9[:, 6:9])
    # eyelid rows: trans diag = epp
    V.tensor_copy(Ab[:, 58 * 12 + 3:58 * 12 + 36:16],
                  epp[:, 1:2].broadcast_to([B, 3]))
    V.tensor_copy(Ab[:, 61 * 12 + 3:61 * 12 + 36:16],
                  epp[:, 0:1].broadcast_to([B, 3]))

    # ---------------- stage A: blend shapes (fp16 v_posed out) ------------
    vp16 = [ptile((128, 384), f"vp{i}", F16) for i in range(NCH)]
    vpf16 = [ptile((128, 384), f"vpf{h}", F16) for h in range(3)]
    vpm16 = [ptile((128, 384), f"vpm{h}", F16) for h in range(2)]

    def stage_a_chunk(i):
        # NB: per PSUM bank only one accumulation group may be open at a
        # time, so each (chunk, c3) group runs start..stop contiguously.
        pq = psum.tile([128, 512], F32, tag="ps", bufs=2)
        pda = pd_a_t[:, i * 384:(i + 1) * 384]
        pdb = pd_b_t[:, i * 384:(i + 1) * 384]
        if i in CH_PLAIN:
            sdt = sd_s_t[i // 2][:, (i % 2) * 1152:(i % 2 + 1) * 1152]
            for c3 in range(3):
                for lk in range(3):
                    T.matmul(pq[:, c3 * 128:(c3 + 1) * 128],
                             sdt[:, (c3 * 3 + lk) * 128:(c3 * 3 + lk + 1) * 128],
                             betaT_s[:, lk * 128:(lk + 1) * 128],
                             start=(lk == 0), stop=False)
                T.matmul(pq[:, c3 * 128:(c3 + 1) * 128],
                         pda[:, c3 * 128:(c3 + 1) * 128], pfT_s_a[:],
                         start=False, stop=False)
                T.matmul(pq[:, c3 * 128:(c3 + 1) * 128],
                         pdb[:, c3 * 128:(c3 + 1) * 128], pfT_s_b[:],
                         start=False, stop=True)
        else:
            for c3 in range(3):
                T.matmul(pq[:, c3 * 128:(c3 + 1) * 128],
                         pda[:, c3 * 128:(c3 + 1) * 128], pfT_s_a[:],
                         start=True, stop=False)
                T.matmul(pq[:, c3 * 128:(c3 + 1) * 128],
                         pdb[:, c3 * 128:(c3 + 1) * 128], pfT_s_b[:],
                         start=False, stop=True)
        S.copy(vp16[i][:], pq[:, 0:384])

    def stage_a_flame(h):
        sdt = sd_f_t[:, h * 1152:(h + 1) * 1152]
        pdf = pd_f_t[:, h * 384:(h + 1) * 384]
        pq = psum.tile([128, 512], F32, tag="ps", bufs=2)
        for c3 in range(3):
            for lk in range(3):
                T.matmul(pq[:, c3 * 128:(c3 + 1) * 128],
                         sdt[:, (c3 * 3 + lk) * 128:(c3 * 3 + lk + 1) * 128],
                         betaT_f[:, lk * 128:(lk + 1) * 128],
                         start=(lk == 0), stop=False)
            T.matmul(pq[:, c3 * 128:(c3 + 1) * 128],
                     pdf[:, c3 * 128:(c3 + 1) * 128], pfT_f[:],
                     start=False, stop=True)
        S.copy(vpf16[h][:], pq[:, 0:384])

    def stage_a_mano(h):
        sdt = sd_m_t[:, h * 384:(h + 1) * 384]
        pq = psum.tile([128, 512], F32, tag="ps", bufs=2)
        for c3 in range(3):
            T.matmul(pq[:, 384 + c3:385 + c3], sdt[:, c3 * 128:(c3 + 1) * 128],
                     betam, start=True, stop=True)
        pda = pd_ma_t[:, h * 384:(h + 1) * 384]
        pdb = pd_mb_t[:, h * 384:(h + 1) * 384]
        for c3 in range(3):
            T.matmul(pq[:, c3 * 128:(c3 + 1) * 128],
                     pda[:, c3 * 128:(c3 + 1) * 128], pfT_m_a[h][:],
                     start=True, stop=False)
            T.matmul(pq[:, c3 * 128:(c3 + 1) * 128],
                     pdb[:, c3 * 128:(c3 + 1) * 128], pfT_m_b[h][:],
                     start=False, stop=True)
        vshm = ptile((128, 3), f"vshm{h}")
        S.copy(vshm[:], pq[:, 384:387])
        for c3 in range(3):
            S.add(vpm16[h][:, c3 * 128:(c3 + 1) * 128],
                  pq[:, c3 * 128:(c3 + 1) * 128], vshm[:, c3:c3 + 1])

    for i in range(NCH_PLAIN - 1):
        stage_a_chunk(i)

    # ---------------- rhs64: 12 transposes [B,64] -> [64,128] -------------
    # (GpSimd cannot read PSUM: copies go to Scalar/Vector only.)
    rhs = ptile((NJ64, 1536), "rhs", F16)
    for n4 in range(4):
        for m3 in range(3):
            pp = psum.tile([128, 512], F32, tag="ps", bufs=2)
            src = A4[:, 0:NJ64, m3, n4]
            T.matmul(pp[0:NJ64, 0:128], src, ident, is_transpose=True,
                     start=True, stop=True)
            dst = rhs[0:NJ64, n4 * 384 + m3 * 128:n4 * 384 + (m3 + 1) * 128]
            if (n4 * 3 + m3) % 3 == 2:
                V.tensor_copy(dst, pp[0:NJ64, 0:128])
            else:
                S.copy(dst, pp[0:NJ64, 0:128])



    # ---------------- skinning ---------------------------------------------
    # apply: out = sum_c tp[c,m]*x[c] + tp[3,m], tree-structured adds.
    def t_apply(eng, dst_ap, tp_ap, x16_ap, tag="pr", dt=F16):
        tp4 = tp_ap.rearrange("p (n m b) -> p n m b", m=3, b=128)
        d3 = dst_ap.rearrange("p (m b) -> p m b", b=128)
        x4 = x16_ap.rearrange("p (c b) -> p c b", b=128).unsqueeze(2)
        pr_t = slabs.tile((128, 1152), dt, tag=tag, bufs=2)
        sc_t = slabs.tile((128, 384), dt, tag=tag + "s", bufs=2)
        pr = pr_t[:].rearrange("p (n m b) -> p n m b", m=3, b=128)
        s3 = sc_t[:].rearrange("p (m b) -> p m b", b=128)
        eng.tensor_mul(pr, tp4[:, 0:3], x4.broadcast_to([128, 3, 3, 128]))
        eng.tensor_add(s3, pr[:, 0], pr[:, 1])
        eng.tensor_add(d3, pr[:, 2], tp4[:, 3])
        eng.tensor_add(d3, d3, s3)

    def skin_mm(wt, nrows):
        tpm = psum.tile([128, 1536], F32, tag="skin", bufs=2)
        for g in range(3):
            T.matmul(tpm[:, g * 512:(g + 1) * 512], wt,
                     rhs[0:nrows, g * 512:(g + 1) * 512], start=True, stop=True)
        return tpm

    def skin_hv(i):
        # flame/mano skinning: fp32 copy + GpSimd apply (slow but off the
        # Vector critical path; issued first so latency hides)
        if CH_HEAD0 <= i < CH_HEAD0 + 3:
            h = i - CH_HEAD0
            tpm = skin_mm(w_f_t[:, h * 128:(h + 1) * 128], NJ64)
            xv = vpf16[h]
        else:
            h = i - CH_HL
            tpm = skin_mm(w_m_t[:, h * 128:(h + 1) * 128], NJ64)
            xv = vpm16[h]
        tp32 = slabs.tile((128, 1536), F32, tag="tpx", bufs=2)
        S.copy(tp32[:], tpm[:])
        hv = slabs.tile((128, 384), F16, tag="hv", bufs=2)
        t_apply(P, hv[:], tp32[:], xv[:], tag="prp")
        P.tensor_add(vp16[i][:], vp16[i][:], hv[:])

    def skin_sx(i):
        tpm = skin_mm(w_s_t[:, i * 128:(i + 1) * 128], 22)
        ot = slabs.tile((128, 384), F16, tag="outt", bufs=3)
        tp16 = slabs.tile((128, 1536), F16, tag="tps", bufs=4)
        S.copy(tp16[:], tpm[:])
        t_apply(V, ot[:], tp16[:], vp16[i][:])
        DMA.dma_start(out_d[i * 128:(i + 1) * 128, :], ot[:])

    def skin_chunk(i):
        if CH_HEAD0 <= i:
            tp16h = slabs.tile((128, 1536), F16, tag="tps", bufs=4)
            if i < CH_HEAD0 + 3:
                h = i - CH_HEAD0
                tpm = skin_mm(w_f_t[:, h * 128:(h + 1) * 128], NJ64)
                xv = vpf16[h]
            else:
                h = i - CH_HL
                tpm = skin_mm(w_m_t[:, h * 128:(h + 1) * 128], NJ64)
                xv = vpm16[h]
            S.copy(tp16h[:], tpm[:])
            hv = slabs.tile((128, 384), F16, tag="hv", bufs=2)
            t_apply(V, hv[:], tp16h[:], xv[:])
            V.tensor_add(vp16[i][:], vp16[i][:], hv[:])
        skin_sx(i)

    stage_a_chunk(NCH_PLAIN - 1)
    skin_chunk(0)
    skin_chunk(1)
    for i in range(NCH_PLAIN, NCH):
        stage_a_chunk(i)
    skin_chunk(2)
    skin_chunk(3)
    for h in range(3):
        stage_a_flame(h)
    skin_chunk(4)
    for h in range(2):
        transpose_to(pfT_m_a[h][:], pf_m[h][:, 0:128])
        transpose_to(pfT_m_b[h][:], pf_m[h][:, 128:PD_M_K])
    for h in range(2):
        stage_a_mano(h)
    skin_chunk(5)
    for i in range(CH_HEAD0, NCH):
        skin_chunk(i)

    if dbg_d is not None:
        dbg = ptile((128, 2560), "dbg")
        G.memset(dbg[:], 0.0)
        V.tensor_copy(dbg[:, 0:768], Ab[:])
        S.copy(dbg[0:NJ64, 768:768 + 1536], rhs[:])
        S.copy(dbg[:, 2304:2304 + 165], jb[:])
        S.copy(dbg[:, 2469:2469 + 15], jfb[:])
        S.copy(dbg[:, 2484:2484 + 9], bias9[:])
        DMA.dma_start(dbg_d[:], dbg[:])

    es.close()


def _rodrigues(nc, aa, rot, ptile, zero_t):
    """Rodrigues for 55 rots: V handles joints 0:25, GpSimd 25:55; the
    sqrt/sin/cos/reciprocal activations run once on Scalar for all joints."""
    V, S, P = nc.vector, nc.scalar, nc.gpsimd
    J = 55
    aa3 = aa.rearrange("p (j k) -> p j k", k=3)
    sq = ptile((B, J), "rg_sq")
    tmp = ptile((B, 2 * J), "rg_tmp")
    eps_t = ptile((B, 1), "rg_eps")
    P.memset(eps_t[:], 1e-8)
    hpi_t = ptile((B, 1), "rg_hpi")
    P.memset(hpi_t[:], float(np.pi / 2))
    ang = ptile((B, J), "rg_ang")
    inv = ptile((B, J), "rg_inv")
    sn = ptile((B, J), "rg_sin")
    co = ptile((B, J), "rg_cos")
    nv = ptile((B, 3 * J), "rg_n")
    u = ptile((B, J), "rg_u")
    un = ptile((B, 3 * J), "rg_un")
    q = ptile((B, 3 * J), "rg_q")
    d = ptile((B, J), "rg_d")
    dd = ptile((B, J), "rg_dd")
    snv = ptile((B, 3 * J), "rg_snv")
    pp_ = ptile((B, 2 * J), "rg_p")
    r4 = rot[:].rearrange("p (j m n) -> p j m n", m=3, n=3)

    t0 = tmp[:, 0:J]
    V.tensor_mul(sq[:], aa3[:, :, 0], aa3[:, :, 0])
    V.tensor_mul(t0, aa3[:, :, 1], aa3[:, :, 1])
    V.tensor_add(sq[:], sq[:], t0)
    V.tensor_mul(t0, aa3[:, :, 2], aa3[:, :, 2])
    V.tensor_add(sq[:], sq[:], t0)

    S.activation(ang[:], sq[:], AF.Sqrt, bias=eps_t[:])
    S.activation(sn[:], ang[:], AF.Sin, bias=zero_t[:])
    S.activation(co[:], ang[:], AF.Sin, bias=hpi_t[:])
    V.reciprocal(inv[:], ang[:])

    for eng, a, b in ((V, 0, 25), (P, 25, J)):
        n_ = b - a
        n3 = nv[:].rearrange("p (j k) -> p j k", k=3)[:, a:b]
        un3 = un[:].rearrange("p (j k) -> p j k", k=3)[:, a:b]
        q3 = q[:].rearrange("p (j k) -> p j k", k=3)[:, a:b]
        s3 = snv[:].rearrange("p (j k) -> p j k", k=3)[:, a:b]
        eng.tensor_mul(n3, aa3[:, a:b],
                       inv[:, a:b].unsqueeze(2).broadcast_to([B, n_, 3]))
        eng.tensor_scalar(u[:, a:b], co[:, a:b], -1.0, 1.0, ALU.mult, ALU.add)
        eng.tensor_mul(un3, n3, u[:, a:b].unsqueeze(2).broadcast_to([B, n_, 3]))
        eng.tensor_mul(q3, un3, n3)
        eng.tensor_add(d[:, a:b], q3[:, :, 0], q3[:, :, 1])
        eng.tensor_add(d[:, a:b], d[:, a:b], q3[:, :, 2])
        eng.tensor_scalar(dd[:, a:b], d[:, a:b], -1.0, 1.0, ALU.mult, ALU.add)
        eng.tensor_mul(s3, n3, sn[:, a:b].unsqueeze(2).broadcast_to([B, n_, 3]))
        for m in range(3):
            eng.tensor_add(r4[:, a:b, m, m], q3[:, :, m], dd[:, a:b])
        p_ = pp_[:, a:b]
        eng.tensor_mul(p_, un3[:, :, 0], n3[:, :, 1])
        eng.tensor_sub(r4[:, a:b, 0, 1], p_, s3[:, :, 2])
        eng.tensor_add(r4[:, a:b, 1, 0], p_, s3[:, :, 2])
        eng.tensor_mul(p_, un3[:, :, 0], n3[:, :, 2])
        eng.tensor_add(r4[:, a:b, 0, 2], p_, s3[:, :, 1])
        eng.tensor_sub(r4[:, a:b, 2, 0], p_, s3[:, :, 1])
        eng.tensor_mul(p_, un3[:, :, 1], n3[:, :, 2])
        eng.tensor_sub(r4[:, a:b, 1, 2], p_, s3[:, :, 0])
        eng.tensor_add(r4[:, a:b, 2, 1], p_, s3[:, :, 0])


# ================================================================ entry

_CACHED = {}
PROFILE = False
DEBUG = False


def _get_nc():
    if "nc" not in _CACHED:
        _CACHED["nc"] = _build_nc()
    return _CACHED["nc"]


def kernel(**inputs):
    in_maps, vid_all = _host_prep(inputs)
    nc = _get_nc()
    res = run_bass_kernel_spmd(nc, in_maps, core_ids=list(range(NCORES)),
                               trace=PROFILE)
    _CACHED["last_res"] = res
    out = np.zeros((B, VS, 3), np.float32)
    for c in range(NCORES):
        o = np.asarray(res.results[c]["out"]).astype(np.float32).reshape(ROWS, 3, B)
        vok = vid_all[c] >= 0
        out[:, vid_all[c][vok], :] = o[vok].transpose(2, 0, 1)
    return out
